# revision 14
# baseline (speedup 1.0000x reference)
"""Trainium2 Bass kernel for nn_Disulfide_net (edge-parallel GNN scatter).

Strategy (8 NeuronCores, SPMD):
  - Host shards atomPairs by sorting each endpoint column and grouping pairs
    into "chunks" of <=F pairs whose endpoint atoms span a <=31-atom window
    (graph-partition sharding + per-chunk halo of at_name bytes).
  - Device phase 1 (p1-sorted layout): per chunk, pack isSG (at_name==16) of
    the 31-atom window into an int32 bitmap held as a per-partition scalar;
    each pair tests its bit via an exponent-trick (1<<w built from float bits)
    -> m1 mask for all 8M pairs at ~1/128 cycle each.
  - Host relays m1 into the p0-sorted layout (pure index permutation).
  - Device phase 2 (p0-sorted layout): m0 bit-test AND m1 -> sulfur mask.
  - Device phase 3 (sparse tail, ~1/1024 of pairs): indirect-DMA gathers of
    per-atom records, dense energy evaluation, duplicate-group totals via
    per-column PE matmuls (host supplies 0/1 group matrices from pure index
    data), then indirect-DMA row writes into per-core atomEnergy /
    residueEnergy partials; host sums the 8 partials on unshard.
"""
import sys
import numpy as np

sys.path.insert(0, "/opt/trn_rl_repo")

N_CORES = 8
N_ATOMS = 500_000
N_PAIRS = 8_000_000
A = 4
N_BATCH, N_CHAIN, N_RES = 2, 4, 50000
SG_HASH = 16
TEMPERATURE = 298.0
WIN = 31          # atoms per chunk window (bit 31 of int32 never used)
F = 640           # pair slots per chunk
_prog_cache = {}


def _build_layout(pvals, T):
    """Assign each pair a slot in the [8, 128, T, F] chunked layout, sorted by
    its endpoint atom id. Returns (slot_of_pair, per-core arrays)."""
    order = np.argsort(pvals, kind="stable")
    ps = pvals[order]
    win = ps // WIN
    uw, first = np.unique(win, return_index=True)
    cnts = np.diff(np.r_[first, len(ps)])
    nchunk_per_win = (cnts + F - 1) // F
    chunk_base = np.r_[0, np.cumsum(nchunk_per_win)][:-1]
    n_chunks = int(chunk_base[-1] + nchunk_per_win[-1]) if len(uw) else 0
    assert n_chunks <= 8 * 128 * T, f"{n_chunks} chunks > capacity {8*128*T}"
    rank = np.arange(len(ps)) - np.repeat(first, cnts)
    chunk_of = np.repeat(chunk_base, cnts) + rank // F
    slot_in = rank % F
    # chunk c -> core = c // (128*T), p = (c % (128*T)) % 128, t = (...) // 128
    c_loc = chunk_of % (128 * T)
    # [core, p, t, f] flat = ((core*128 + p)*T + t)*F + f
    slot = ((chunk_of // (128 * T)) * 128 + (c_loc % 128)) * (T * F) \
        + (c_loc // 128) * F + slot_in
    slot_of_pair = np.empty(len(ps), np.int64)
    slot_of_pair[order] = slot
    # per-chunk metadata, scattered into the [core, p, t] layout order
    cid = np.arange(n_chunks)
    cl = cid % (128 * T)
    lay = ((cid // (128 * T)) * 128 + (cl % 128)) * T + (cl // 128)
    base_of_chunk = np.zeros(8 * 128 * T, np.int64)
    base_of_chunk[lay] = np.repeat(uw, nchunk_per_win) * WIN
    pf = np.zeros((8, 128, T, F), np.float32)
    pf.reshape(-1)[slot] = ps  # pad slots keep 0 -> w may alias bit0; killed by mfac
    # pads must stay in-window: set pad slots' value to the chunk's base
    padmask = np.ones(8 * 128 * T * F, bool)
    padmask[slot] = False
    cb = np.repeat(base_of_chunk, F)
    pfflat = pf.reshape(-1)
    pfflat[padmask] = cb[padmask]
    basem = (base_of_chunk.astype(np.float32) - 127.0)
    basem = basem.reshape(8, 128, T)
    return slot_of_pair, pf, basem, base_of_chunk.reshape(8, 128, T), n_chunks


def _build_anw(at_name_u8, base_of_chunk):
    """Per-chunk 32-byte at_name halo windows. [8,128,T,32] int8."""
    idx = base_of_chunk[..., None] + np.arange(32)
    valid = idx < N_ATOMS
    idxc = np.clip(idx, 0, N_ATOMS - 1)
    anw = at_name_u8[idxc]
    anw[~valid] = 0
    anw[..., 31] = 0  # bit31 unused
    return anw.astype(np.int8)


def _build_program(T):
    import concourse.bacc as bacc
    import concourse.mybir as mybir
    from concourse.tile import TileContext
    dt = mybir.dt
    AluOp = mybir.AluOpType

    nc = bacc.Bacc("TRN2", target_bir_lowering=False, debug=False,
                   num_devices=N_CORES)
    TF = T * F
    pf_d = nc.dram_tensor("pf", [128, TF], dt.float32, kind="ExternalInput")
    basem_d = nc.dram_tensor("basem", [128, T], dt.float32, kind="ExternalInput")
    anw_d = nc.dram_tensor("anw", [128, T * 32], dt.int8, kind="ExternalInput")
    wts_d = nc.dram_tensor("wts", [128, T * 32], dt.float32, kind="ExternalInput")
    mfac_d = nc.dram_tensor("mfac", [128, TF], dt.float32, kind="ExternalInput")
    mby_d = nc.dram_tensor("mby", [128, TF], dt.int8, kind="ExternalOutput")
    mfl_d = nc.dram_tensor("mfl", [128, TF], dt.float32, kind="ExternalOutput")

    with TileContext(nc) as tc:
        with tc.tile_pool(name="big", bufs=1) as big, \
             tc.tile_pool(name="small", bufs=2) as small:
            pf = big.tile([128, TF], dt.float32)
            mfac = big.tile([128, TF], dt.float32)
            mfl = big.tile([128, TF], dt.float32)
            mby = big.tile([128, TF], dt.int8)
            basem = big.tile([128, T], dt.float32)
            anw = big.tile([128, T * 32], dt.int8)
            wts = big.tile([128, T * 32], dt.float32)
            nc.sync.dma_start(pf[:], pf_d.ap())
            nc.sync.dma_start(basem[:], basem_d.ap())
            nc.sync.dma_start(anw[:], anw_d.ap())
            nc.sync.dma_start(wts[:], wts_d.ap())
            nc.sync.dma_start(mfac[:], mfac_d.ap())

            # ---- bitmap build: B[p, t] int32 of isSG over the chunk window
            anf = big.tile([128, T * 32], dt.float32)
            nc.vector.tensor_copy(anf[:], anw[:])
            eq = big.tile([128, T * 32], dt.float32)
            nc.vector.tensor_scalar(eq[:], anf[:], float(SG_HASH), None,
                                    AluOp.is_equal)
            nc.vector.tensor_tensor(eq[:], eq[:], wts[:], AluOp.mult)
            eq3 = eq[:].rearrange("p (t k) -> p t k", k=32)
            lo = big.tile([128, T], dt.float32)
            hi = big.tile([128, T], dt.float32)
            nc.vector.tensor_reduce(lo[:], eq3[:, :, 0:16],
                                    axis=mybir.AxisListType.X,
                                    op=AluOp.add)
            nc.vector.tensor_reduce(hi[:], eq3[:, :, 16:32],
                                    axis=mybir.AxisListType.X,
                                    op=AluOp.add)
            loi = big.tile([128, T], dt.int32)
            hii = big.tile([128, T], dt.int32)
            nc.vector.tensor_copy(loi[:], lo[:])
            nc.vector.tensor_copy(hii[:], hi[:])
            nc.vector.tensor_scalar(hii[:], hii[:], 16, None,
                                    AluOp.logical_shift_left)
            B = big.tile([128, T], dt.int32)
            nc.vector.tensor_tensor(B[:], loi[:], hii[:], AluOp.bitwise_or)

            # ---- per-tile mask
            for t in range(T):
                sl = slice(t * F, (t + 1) * F)
                wf = small.tile([128, F], dt.float32, tag="wf")
                ei = small.tile([128, F], dt.int32, tag="ei")
                pw = small.tile([128, F], dt.int32, tag="pw")
                mf = small.tile([128, F], dt.float32, tag="mf")
                nc.vector.tensor_scalar(wf[:], pf[:, sl], basem[:, t:t + 1],
                                        None, AluOp.subtract)
                nc.vector.tensor_copy(ei[:], wf[:])
                nc.vector.tensor_scalar(ei[:], ei[:], 23, None,
                                        AluOp.logical_shift_left)
                nc.vector.tensor_copy(pw[:], ei[:].bitcast(dt.float32))
                nc.vector.tensor_scalar(pw[:], pw[:], B[:, t:t + 1], None,
                                        AluOp.bitwise_and)
                nc.vector.tensor_copy(mf[:], pw[:])
                nc.vector.tensor_tensor(mf[:], mf[:], mfac[:, sl], AluOp.mult)
                nc.vector.tensor_scalar(mfl[:, sl], mf[:], 0.0, None,
                                        AluOp.not_equal)
                nc.vector.tensor_copy(mby[:, sl], mfl[:, sl])
            nc.sync.dma_start(mby_d.ap(), mby[:])
            nc.sync.dma_start(mfl_d.ap(), mfl[:])
    nc.compile()
    return nc


def _get_program(T):
    if T not in _prog_cache:
        _prog_cache[T] = _build_program(T)
    return _prog_cache[T]


def _run_phase(nc, pf, basem, anw, wts, mfac):
    from concourse import bass_utils
    in_maps = [
        dict(pf=np.ascontiguousarray(pf[c].reshape(128, -1)),
             basem=np.ascontiguousarray(basem[c]),
             anw=np.ascontiguousarray(anw[c].reshape(128, -1)),
             wts=wts,
             mfac=np.ascontiguousarray(mfac[c].reshape(128, -1)))
        for c in range(N_CORES)
    ]
    res = bass_utils.run_bass_kernel_spmd(nc, in_maps,
                                          core_ids=list(range(N_CORES)))
    return res


C_PAIR = 8            # pair columns per core in the tail (capacity 1024 pairs)
REC_P = 3936          # 128*3936 = 503808 >= N_ATOMS
NBINS = N_BATCH * N_CHAIN * N_RES


def _build_tail_program():
    import concourse.bacc as bacc
    import concourse.mybir as mybir
    from concourse.tile import TileContext
    dt = mybir.dt
    AluOp = mybir.AluOpType
    AF = mybir.ActivationFunctionType
    from concourse.bass import IndirectOffsetOnAxis

    C = C_PAIR
    nc = bacc.Bacc("TRN2", target_bir_lowering=False, debug=False,
                   num_devices=N_CORES)
    co_d = nc.dram_tensor("co", [128, REC_P * 3], dt.float32, kind="ExternalInput")
    adf_d = nc.dram_tensor("adf", [128, REC_P * 3], dt.float32, kind="ExternalInput")
    alb_d = nc.dram_tensor("alb", [128, REC_P], dt.float32, kind="ExternalInput")
    p0i_d = nc.dram_tensor("p0i", [128, C], dt.int32, kind="ExternalInput")
    p1i_d = nc.dram_tensor("p1i", [128, C], dt.int32, kind="ExternalInput")
    p0f_d = nc.dram_tensor("p0f", [128, C], dt.float32, kind="ExternalInput")
    ma0_d = nc.dram_tensor("ma0", [128, C * 128], dt.float32, kind="ExternalInput")
    ma1_d = nc.dram_tensor("ma1", [128, C * 128], dt.float32, kind="ExternalInput")
    mr0_d = nc.dram_tensor("mr0", [128, C * 128], dt.float32, kind="ExternalInput")
    mr1_d = nc.dram_tensor("mr1", [128, C * 128], dt.float32, kind="ExternalInput")
    prow_d = nc.dram_tensor("prow", [128, 1], dt.float32, kind="ExternalInput")
    ae_d = nc.dram_tensor("ae", [N_ATOMS + 128, A], dt.float32, kind="ExternalOutput")
    re_d = nc.dram_tensor("re", [NBINS + 128, A], dt.float32, kind="ExternalOutput")
    dbg_d = nc.dram_tensor("dbg", [128, C * 24], dt.float32, kind="ExternalOutput")
    dbg2_d = nc.dram_tensor("dbg2", [1, 128], dt.float32, kind="ExternalOutput")
    rec_t = nc.dram_tensor("rectab", [128 * REC_P, 8], dt.float32, kind="Internal")

    with TileContext(nc) as tc:
        with tc.tile_pool(name="big", bufs=1) as big, \
             tc.tile_pool(name="sm", bufs=2) as sm, \
             tc.tile_pool(name="ps", bufs=2, space="PSUM") as ps:
            rec_dram = rec_t
            # ---- build fat record table [atom, 8] = x,y,z,resnum,flat,altbits,0,0
            NSL = 4
            SL = REC_P // NSL
            for s in range(NSL):
                co = sm.tile([128, SL, 3], dt.float32, tag="co")
                adf = sm.tile([128, SL, 3], dt.float32, tag="adf")
                alb = sm.tile([128, SL], dt.float32, tag="alb")
                rec = sm.tile([128, SL, 8], dt.float32, tag="rec")
                nc.sync.dma_start(co[:], co_d.ap().rearrange(
                    "p (n c) -> p n c", c=3)[:, s * SL:(s + 1) * SL, :])
                nc.sync.dma_start(adf[:], adf_d.ap().rearrange(
                    "p (n c) -> p n c", c=3)[:, s * SL:(s + 1) * SL, :])
                nc.sync.dma_start(alb[:], alb_d.ap()[:, s * SL:(s + 1) * SL])
                for k in range(3):
                    nc.vector.tensor_copy(rec[:, :, k], co[:, :, k])
                nc.vector.tensor_copy(rec[:, :, 3], adf[:, :, 2])
                # flat = b*200000 + c*50000 + r
                nc.vector.tensor_scalar(rec[:, :, 4], adf[:, :, 0], 200000.0,
                                        None, AluOp.mult)
                fl2 = sm.tile([128, SL], dt.float32, tag="fl2")
                nc.vector.tensor_scalar(fl2[:], adf[:, :, 1], 50000.0, None,
                                        AluOp.mult)
                nc.vector.tensor_tensor(rec[:, :, 4], rec[:, :, 4], fl2[:],
                                        AluOp.add)
                nc.vector.tensor_tensor(rec[:, :, 4], rec[:, :, 4],
                                        adf[:, :, 2], AluOp.add)
                nc.vector.tensor_copy(rec[:, :, 5], alb[:])
                nc.gpsimd.memset(rec[:, :, 6:8], 0.0)
                nc.sync.dma_start(
                    rec_dram.ap().rearrange("(p n) e -> p n e", p=128)
                    [:, s * SL:(s + 1) * SL, :], rec[:])

            dbg2 = big.tile([1, 128], dt.float32)
            nc.sync.dma_start(dbg2[:], rec_t.ap()[0:16, :].rearrange("r e -> () (r e)"))
            nc.sync.dma_start(dbg2_d.ap(), dbg2[:])
            p0i = big.tile([128, C], dt.int32)
            p1i = big.tile([128, C], dt.int32)
            p0f = big.tile([128, C], dt.float32)
            prow_d_col = big.tile([128, 1], dt.float32)
            nc.sync.dma_start(p0i[:], p0i_d.ap())
            nc.sync.dma_start(p1i[:], p1i_d.ap())
            nc.sync.dma_start(p0f[:], p0f_d.ap())
            nc.sync.dma_start(prow_d_col[:], prow_d.ap())
            rec0f = big.tile([128, C * 8], dt.float32)
            rec1f = big.tile([128, C * 8], dt.float32)
            nc.gpsimd.memset(rec0f[:], 0.0)
            nc.gpsimd.memset(rec1f[:], 0.0)
            for c in range(C):
                nc.gpsimd.indirect_dma_start(
                    rec0f[:, c * 8:(c + 1) * 8], None, rec_dram.ap(),
                    IndirectOffsetOnAxis(ap=p0i[:, c:c + 1], axis=0))
                nc.gpsimd.indirect_dma_start(
                    rec1f[:, c * 8:(c + 1) * 8], None, rec_dram.ap(),
                    IndirectOffsetOnAxis(ap=p1i[:, c:c + 1], axis=0))
            rec0 = rec0f[:].rearrange("p (c e) -> p c e", e=8)
            rec1 = rec1f[:].rearrange("p (c e) -> p c e", e=8)

            # ---- energy per pair slot [128, C]
            d2 = big.tile([128, C], dt.float32)
            tmp = big.tile([128, C], dt.float32)
            nc.gpsimd.memset(d2[:], 0.0)
            for k in range(3):
                dx = sm.tile([128, C], dt.float32, tag="dx")
                nc.vector.tensor_tensor(dx[:], rec0[:, :, k], rec1[:, :, k],
                                        AluOp.subtract)
                nc.vector.tensor_scalar(dx[:], dx[:], 1e-6, None, AluOp.add)
                nc.vector.tensor_tensor(dx[:], dx[:], dx[:], AluOp.mult)
                nc.vector.tensor_tensor(d2[:], d2[:], dx[:], AluOp.add)
            dist = big.tile([128, C], dt.float32)
            nc.scalar.activation(dist[:], d2[:], mybir.ActivationFunctionType.Sqrt)
            padv01 = big.tile([128, C], dt.float32)
            nc.vector.tensor_scalar(padv01[:], p0f[:], float(N_ATOMS) - 0.5,
                                    None, AluOp.is_gt)
            rd = big.tile([128, C], dt.float32)
            nc.vector.tensor_tensor(rd[:], rec0[:, :, 3], rec1[:, :, 3],
                                    AluOp.subtract)
            nc.scalar.activation(rd[:], rd[:], mybir.ActivationFunctionType.Abs)
            nc.vector.tensor_tensor(rd[:], rd[:], padv01[:], AluOp.add)
            lg = big.tile([128, C], dt.float32)
            nc.scalar.activation(lg[:], rd[:], mybir.ActivationFunctionType.Ln)
            netE = big.tile([128, C], dt.float32)
            # netE = 0.5*(-0.298*(2.1 + 2.9823825*lg) + 5*|dist-2.04|)
            nc.vector.tensor_scalar(netE[:], lg[:],
                                    0.5 * -0.001 * TEMPERATURE * 2.9823825,
                                    0.5 * -0.001 * TEMPERATURE * 2.1,
                                    AluOp.mult, AluOp.add)
            nc.vector.tensor_scalar(tmp[:], dist[:], 2.04, None,
                                    AluOp.subtract)
            nc.scalar.activation(tmp[:], tmp[:], mybir.ActivationFunctionType.Abs)
            nc.vector.tensor_scalar(tmp[:], tmp[:], 2.5, None, AluOp.mult)
            nc.vector.tensor_tensor(netE[:], netE[:], tmp[:], AluOp.add)
            # alt bits -> contrib [128, C, 4]
            ab = big.tile([128, C], dt.int32)
            a0i = big.tile([128, C], dt.int32)
            a1i = big.tile([128, C], dt.int32)
            nc.vector.tensor_copy(a0i[:], rec0[:, :, 5])
            nc.vector.tensor_copy(a1i[:], rec1[:, :, 5])
            nc.vector.tensor_tensor(ab[:], a0i[:], a1i[:], AluOp.bitwise_and)
            contrib = big.tile([128, C, 4], dt.float32)
            for k in range(A):
                bk = sm.tile([128, C], dt.int32, tag="bk")
                bf = sm.tile([128, C], dt.float32, tag="bf")
                nc.vector.tensor_scalar(bk[:], ab[:], 1 << k, None,
                                        AluOp.bitwise_and)
                nc.vector.tensor_copy(bf[:], bk[:])
                nc.vector.tensor_scalar(bf[:], bf[:], 0.0, None, AluOp.not_equal)
                nc.vector.tensor_tensor(contrib[:, :, k], netE[:], bf[:],
                                        AluOp.mult)

            # ---- flat offsets (+pad fix: pads have p0f > N_ATOMS)
            # pads write to dump rows NBINS+p (sliced off on host)
            prow = big.tile([128, C], dt.float32)
            nc.vector.tensor_scalar(prow[:], padv01[:], prow_d_col[:], None,
                                    AluOp.mult)
            vm = big.tile([128, C], dt.float32)
            nc.vector.tensor_scalar(vm[:], padv01[:], -1.0, 1.0, AluOp.mult,
                                    AluOp.add)
            f0 = big.tile([128, C], dt.float32)
            f1 = big.tile([128, C], dt.float32)
            nc.vector.tensor_tensor(f0[:], rec0[:, :, 4], vm[:], AluOp.mult)
            nc.vector.tensor_tensor(f0[:], f0[:], prow[:], AluOp.add)
            nc.vector.tensor_tensor(f1[:], rec1[:, :, 4], vm[:], AluOp.mult)
            nc.vector.tensor_tensor(f1[:], f1[:], prow[:], AluOp.add)
            f0i = big.tile([128, C], dt.int32)
            f1i = big.tile([128, C], dt.int32)
            nc.vector.tensor_copy(f0i[:], f0[:])
            nc.vector.tensor_copy(f1i[:], f1[:])

            # ---- dedup totals via per-column group matmuls, then write scatters
            ma0 = big.tile([128, C * 128], dt.float32)
            ma1 = big.tile([128, C * 128], dt.float32)
            mr0 = big.tile([128, C * 128], dt.float32)
            mr1 = big.tile([128, C * 128], dt.float32)
            nc.sync.dma_start(ma0[:], ma0_d.ap())
            nc.sync.dma_start(ma1[:], ma1_d.ap())
            nc.sync.dma_start(mr0[:], mr0_d.ap())
            nc.sync.dma_start(mr1[:], mr1_d.ap())
            dbg = big.tile([128, C * 24], dt.float32)
            nc.vector.tensor_copy(dbg[:].rearrange("p (c e) -> p c e", e=24)[:, :, 0:8], rec0)
            nc.vector.tensor_copy(dbg[:].rearrange("p (c e) -> p c e", e=24)[:, :, 8:16], rec1)
            nc.vector.tensor_copy(dbg[:].rearrange("p (c e) -> p c e", e=24)[:, :, 16:17], netE[:].rearrange("p c -> p c ()"))
            nc.vector.tensor_copy(dbg[:].rearrange("p (c e) -> p c e", e=24)[:, :, 17:21], contrib[:])
            nc.vector.tensor_copy(dbg[:].rearrange("p (c e) -> p c e", e=24)[:, :, 21:22], rd[:].rearrange("p c -> p c ()"))
            nc.vector.tensor_copy(dbg[:].rearrange("p (c e) -> p c e", e=24)[:, :, 22:23], dist[:].rearrange("p c -> p c ()"))
            nc.sync.dma_start(dbg_d.ap(), dbg[:])
            tots = {}
            for name, m in (("a0", ma0), ("a1", ma1), ("r0", mr0), ("r1", mr1)):
                tot = big.tile([128, C * 4], dt.float32, tag="tot" + name)
                for c in range(C):
                    pt = ps.tile([128, 4], dt.float32, tag="pt")
                    nc.tensor.matmul(out=pt[:], lhsT=m[:, c * 128:(c + 1) * 128],
                                     rhs=contrib[:, c, :], start=True, stop=True)
                    nc.vector.tensor_copy(tot[:, c * 4:(c + 1) * 4], pt[:])
                tots[name] = tot
            for c in range(C):
                nc.gpsimd.indirect_dma_start(
                    ae_d.ap(), IndirectOffsetOnAxis(ap=p0i[:, c:c + 1], axis=0),
                    tots["a0"][:, c * 4:(c + 1) * 4], None)
                nc.gpsimd.indirect_dma_start(
                    ae_d.ap(), IndirectOffsetOnAxis(ap=p1i[:, c:c + 1], axis=0),
                    tots["a1"][:, c * 4:(c + 1) * 4], None)
                nc.gpsimd.indirect_dma_start(
                    re_d.ap(), IndirectOffsetOnAxis(ap=f0i[:, c:c + 1], axis=0),
                    tots["r0"][:, c * 4:(c + 1) * 4], None)
                nc.gpsimd.indirect_dma_start(
                    re_d.ap(), IndirectOffsetOnAxis(ap=f1i[:, c:c + 1], axis=0),
                    tots["r1"][:, c * 4:(c + 1) * 4], None)
    nc.compile()
    return nc


def _get_tail_program():
    if "tail" not in _prog_cache:
        _prog_cache["tail"] = _build_tail_program()
    return _prog_cache["tail"]


class _UF:
    def __init__(self, n):
        self.p = list(range(n))

    def find(self, x):
        while self.p[x] != x:
            self.p[x] = self.p[self.p[x]]
            x = self.p[x]
        return x

    def union(self, a, b):
        ra, rb = self.find(a), self.find(b)
        if ra != rb:
            self.p[ra] = rb


def _pack_tail_core(h0, h1, fl0, fl1):
    """Pack this core's hit pairs into [128, C_PAIR] columns such that no two
    columns share an atom id or flat id. Returns (rows, cols, ok)."""
    n = len(h0)
    C = C_PAIR
    if n == 0:
        return np.zeros(0, np.int64), np.zeros(0, np.int64), True
    uf = _UF(n)
    for keys in (np.concatenate([h0, h1]), np.concatenate([fl0, fl1])):
        pid = np.tile(np.arange(n), 2)
        o = np.argsort(keys, kind="stable")
        ks, po = keys[o], pid[o]
        same = np.flatnonzero(ks[1:] == ks[:-1])
        for i in same:
            uf.union(int(po[i]), int(po[i + 1]))
    root = np.array([uf.find(i) for i in range(n)])
    comps = {}
    for i, r in enumerate(root):
        comps.setdefault(r, []).append(i)
    # first-fit into C columns of capacity 128
    cap = [128] * C
    rows = np.empty(n, np.int64)
    cols = np.empty(n, np.int64)
    for members in sorted(comps.values(), key=len, reverse=True):
        sz = len(members)
        if sz > 128:
            return rows, cols, False
        placed = False
        for c in range(C):
            if cap[c] >= sz:
                r0 = 128 - cap[c]
                for j, m in enumerate(members):
                    rows[m] = r0 + j
                    cols[m] = c
                cap[c] -= sz
                placed = True
                break
        if not placed:
            return rows, cols, False
    return rows, cols, True


def _host_tail(coords, resnum, alt, p0, p1, hit):
    atomEnergy = np.zeros((N_ATOMS, A), np.float32)
    if len(hit):
        h0, h1 = p0[hit], p1[hit]
        diff = coords[h0] - coords[h1] + 1e-6
        dist = np.sqrt(np.sum(diff * diff, axis=-1))
        rd = np.abs(resnum[h0] - resnum[h1]).astype(np.float32)
        energy = -0.001 * TEMPERATURE * (2.1 + 2.9823825 * np.log(rd)) \
            + 5.0 * np.abs(dist - 2.04)
        netE = 0.5 * energy
        pair_alt = alt[h0] & alt[h1]
        contrib = np.where(pair_alt, netE[:, None], 0.0).astype(np.float32)
        np.add.at(atomEnergy, h0, contrib)
        np.add.at(atomEnergy, h1, contrib)
    return atomEnergy


def kernel(coords, atom_description, atom_number, atomPairs, alternativeMask,
           partners, facc):
    coords = np.asarray(coords, np.float32)
    ad = np.asarray(atom_description)
    pairs = np.asarray(atomPairs).astype(np.int64)
    alt = np.asarray(alternativeMask).astype(bool)
    at_name = ad[:, 3].astype(np.int64)
    resnum = ad[:, 2].astype(np.int64)
    p0, p1 = pairs[:, 0], pairs[:, 1]
    npair = len(p0)

    at_u8 = at_name.astype(np.uint8)

    # choose T adaptively (worst-case chunk count over both sides)
    def n_chunks_needed(p):
        win_counts = np.bincount(p // WIN)
        wc = win_counts[win_counts > 0]
        return int(np.sum((wc + F - 1) // F))

    need = max(n_chunks_needed(p0), n_chunks_needed(p1))
    T = max(1, -(-need // (8 * 128)))
    nc = _get_program(T)

    # weights for bitmap packing
    k = np.arange(32)
    w_lo = np.where(k < 16, 2.0 ** k, 0.0)
    w_hi = np.where(k >= 16, 2.0 ** (k - 16), 0.0)
    wts = np.tile((w_lo + w_hi).astype(np.float32), T)[None, :].repeat(128, 0)
    wts[:, 31::32] = 0.0

    # ---- phase 1: p1 side
    slot1, pf1, basem1, boc1, _ = _build_layout(p1, T)
    anw1 = _build_anw(at_u8, boc1)
    ones = np.ones((8, 128, T * F), np.float32)
    res1 = _run_phase(nc, pf1, basem1, anw1, wts, ones)
    m1 = np.stack([res1.results[c]["mfl"] for c in range(N_CORES)])  # [8,128,TF]
    m1_of_pair = m1.reshape(-1)[slot1]

    # ---- relay into p0 layout
    slot0, pf0, basem0, boc0, _ = _build_layout(p0, T)
    anw0 = _build_anw(at_u8, boc0)
    mfac0 = np.zeros(8 * 128 * T * F, np.float32)
    mfac0[slot0] = m1_of_pair
    res0 = _run_phase(nc, pf0, basem0, anw0, wts, mfac0.reshape(8, 128, T * F))
    sby = np.stack([res0.results[c]["mby"] for c in range(N_CORES)])
    sulfur = sby.reshape(-1)[slot0] != 0

    # ---- sparse tail on device (phase 3); host fallback for over-capacity
    flat = ((ad[:, 0].astype(np.int64) * N_CHAIN + ad[:, 1]) * N_RES + resnum)
    hit = np.flatnonzero(sulfur)
    # Device tail by default; host fallback only for over-capacity inputs
    # (>128*C_PAIR sulfur pairs per core) or if explicitly disabled.
    import os
    use_device_tail = (os.environ.get("DISULF_DEVICE_TAIL", "1") == "1"
                       and len(hit) <= N_CORES * 128 * C_PAIR)

    if use_device_tail:
        from concourse import bass_utils
        nct = _get_tail_program()
        C = C_PAIR
        # shared per-atom inputs, padded to 128*REC_P
        NP_ = 128 * REC_P
        co_pad = np.zeros((NP_, 3), np.float32)
        co_pad[:N_ATOMS] = coords
        adf_pad = np.zeros((NP_, 3), np.float32)
        adf_pad[:N_ATOMS, 0] = ad[:, 0]
        adf_pad[:N_ATOMS, 1] = ad[:, 1]
        adf_pad[:N_ATOMS, 2] = resnum
        alb_pad = np.zeros(NP_, np.float32)
        alb_pad[:N_ATOMS] = (alt * (1 << np.arange(4))).sum(1)
        co_in = co_pad.reshape(128, REC_P * 3)
        adf_in = adf_pad.reshape(128, REC_P * 3)
        alb_in = alb_pad.reshape(128, REC_P)
        # split hits evenly across cores
        in_maps = []
        per = [hit[c::N_CORES] for c in range(N_CORES)]
        ok_all = True
        for c in range(N_CORES):
            hc = per[c]
            h0, h1 = p0[hc], p1[hc]
            f0, f1 = flat[h0], flat[h1]
            rows, cols, ok = _pack_tail_core(h0, h1, f0, f1)
            ok_all &= ok
            if not ok:
                break
            rowpad = (N_ATOMS + np.arange(128, dtype=np.int32))[:, None]
            p0i = np.broadcast_to(rowpad, (128, C)).astype(np.int32).copy()
            p1i = p0i.copy()
            # pad ids N_ATOMS+p: never equal a real atom; within a column all distinct
            a0 = p0i.astype(np.int64).copy()
            a1 = a0.copy()
            g0 = np.broadcast_to(NBINS + np.arange(128)[:, None],
                                 (128, C)).astype(np.int64).copy()
            g1 = g0.copy()
            a0[rows, cols] = h0
            a1[rows, cols] = h1
            g0[rows, cols] = f0
            g1[rows, cols] = f1
            p0i[rows, cols] = h0
            p1i[rows, cols] = h1
            ma0 = np.zeros((128, C, 128), np.float32)
            ma1 = np.zeros((128, C, 128), np.float32)
            mr0 = np.zeros((128, C, 128), np.float32)
            mr1 = np.zeros((128, C, 128), np.float32)
            for cc in range(C):
                A0, A1 = a0[:, cc], a1[:, cc]
                G0, G1 = g0[:, cc], g1[:, cc]
                ma0[:, cc, :] = ((A0[:, None] == A0[None, :]) +
                                 (A1[:, None] == A0[None, :])).astype(np.float32)
                ma1[:, cc, :] = ((A0[:, None] == A1[None, :]) +
                                 (A1[:, None] == A1[None, :])).astype(np.float32)
                mr0[:, cc, :] = ((G0[:, None] == G0[None, :]) +
                                 (G1[:, None] == G0[None, :])).astype(np.float32)
                mr1[:, cc, :] = ((G0[:, None] == G1[None, :]) +
                                 (G1[:, None] == G1[None, :])).astype(np.float32)
            in_maps.append(dict(
                co=co_in, adf=adf_in, alb=alb_in,
                p0i=p0i, p1i=p1i, p0f=p0i.astype(np.float32),
                ma0=ma0.reshape(128, -1), ma1=ma1.reshape(128, -1),
                mr0=mr0.reshape(128, -1), mr1=mr1.reshape(128, -1),
                prow=(NBINS + np.arange(128, dtype=np.float32))[:, None]))
        if ok_all:
            rest = bass_utils.run_bass_kernel_spmd(
                nct, in_maps, core_ids=list(range(N_CORES)))
            atomEnergy = np.zeros((N_ATOMS, A), np.float32)
            residueEnergy = np.zeros((NBINS, A), np.float32)
            for c in range(N_CORES):
                atomEnergy += rest.results[c]["ae"][:N_ATOMS]
                residueEnergy += rest.results[c]["re"][:NBINS]
        else:
            use_device_tail = False

    if not use_device_tail:
        atomEnergy = _host_tail(coords, resnum, alt, p0, p1, hit)
        residueEnergy = np.zeros((NBINS, A), np.float32)
        nz = np.flatnonzero(np.any(atomEnergy != 0, axis=1))
        np.add.at(residueEnergy, flat[nz], atomEnergy[nz])

    residueEnergy = residueEnergy.reshape(N_BATCH, N_CHAIN, N_RES, A)
    return residueEnergy, atomEnergy, sulfur


# revision 16
# speedup vs baseline: 1.3421x; 1.3421x over previous
"""Trainium2 Bass kernel for nn_Disulfide_net (edge-parallel GNN scatter).

Strategy (8 NeuronCores, SPMD):
  - Host shards atomPairs by sorting each endpoint column and grouping pairs
    into "chunks" of <=F pairs whose endpoint atoms span a <=31-atom window
    (graph-partition sharding + per-chunk halo of at_name bytes).
  - Device phase 1 (p1-sorted layout): per chunk, pack isSG (at_name==16) of
    the 31-atom window into an int32 bitmap held as a per-partition scalar;
    each pair tests its bit via an exponent-trick (1<<w built from float bits)
    -> m1 mask for all 8M pairs at ~1/128 cycle each.
  - Host relays m1 into the p0-sorted layout (pure index permutation).
  - Device phase 2 (p0-sorted layout): m0 bit-test AND m1 -> sulfur mask.
  - Device phase 3 (sparse tail, ~1/1024 of pairs): indirect-DMA gathers of
    per-atom records, dense energy evaluation, duplicate-group totals via
    per-column PE matmuls (host supplies 0/1 group matrices from pure index
    data), then indirect-DMA row writes into per-core atomEnergy /
    residueEnergy partials; host sums the 8 partials on unshard.
"""
import sys
import numpy as np

sys.path.insert(0, "/opt/trn_rl_repo")

N_CORES = 8
N_ATOMS = 500_000
N_PAIRS = 8_000_000
A = 4
N_BATCH, N_CHAIN, N_RES = 2, 4, 50000
SG_HASH = 16
TEMPERATURE = 298.0
WIN = 31          # atoms per chunk window (bit 31 of int32 never used)
F = 640           # pair slots per chunk
_prog_cache = {}


def _build_layout(pvals, T):
    """Assign each pair a slot in the [8, 128, T, F] chunked layout, sorted by
    its endpoint atom id. Returns (slot_of_pair, per-core arrays)."""
    order = np.argsort(pvals, kind="stable")
    ps = pvals[order]
    win = ps // WIN
    uw, first = np.unique(win, return_index=True)
    cnts = np.diff(np.r_[first, len(ps)])
    nchunk_per_win = (cnts + F - 1) // F
    chunk_base = np.r_[0, np.cumsum(nchunk_per_win)][:-1]
    n_chunks = int(chunk_base[-1] + nchunk_per_win[-1]) if len(uw) else 0
    assert n_chunks <= 8 * 128 * T, f"{n_chunks} chunks > capacity {8*128*T}"
    rank = np.arange(len(ps)) - np.repeat(first, cnts)
    chunk_of = np.repeat(chunk_base, cnts) + rank // F
    slot_in = rank % F
    # chunk c -> core = c // (128*T), p = (c % (128*T)) % 128, t = (...) // 128
    c_loc = chunk_of % (128 * T)
    # [core, p, t, f] flat = ((core*128 + p)*T + t)*F + f
    slot = ((chunk_of // (128 * T)) * 128 + (c_loc % 128)) * (T * F) \
        + (c_loc // 128) * F + slot_in
    slot_of_pair = np.empty(len(ps), np.int64)
    slot_of_pair[order] = slot
    # per-chunk metadata, scattered into the [core, p, t] layout order
    cid = np.arange(n_chunks)
    cl = cid % (128 * T)
    lay = ((cid // (128 * T)) * 128 + (cl % 128)) * T + (cl // 128)
    base_of_chunk = np.zeros(8 * 128 * T, np.int64)
    base_of_chunk[lay] = np.repeat(uw, nchunk_per_win) * WIN
    pf = np.zeros((8, 128, T, F), np.float32)
    pf.reshape(-1)[slot] = ps  # pad slots keep 0 -> w may alias bit0; killed by mfac
    # pads must stay in-window: set pad slots' value to the chunk's base
    padmask = np.ones(8 * 128 * T * F, bool)
    padmask[slot] = False
    cb = np.repeat(base_of_chunk, F)
    pfflat = pf.reshape(-1)
    pfflat[padmask] = cb[padmask]
    basem = (base_of_chunk.astype(np.float32) - 127.0)
    basem = basem.reshape(8, 128, T)
    return slot_of_pair, pf, basem, base_of_chunk.reshape(8, 128, T), n_chunks


def _build_anw(at_name_u8, base_of_chunk):
    """Per-chunk 32-byte at_name halo windows. [8,128,T,32] int8."""
    idx = base_of_chunk[..., None] + np.arange(32)
    valid = idx < N_ATOMS
    idxc = np.clip(idx, 0, N_ATOMS - 1)
    anw = at_name_u8[idxc]
    anw[~valid] = 0
    anw[..., 31] = 0  # bit31 unused
    return anw.astype(np.int8)


def _build_program(T, phase):
    import concourse.bacc as bacc
    import concourse.mybir as mybir
    from concourse.tile import TileContext
    dt = mybir.dt
    AluOp = mybir.AluOpType

    nc = bacc.Bacc("TRN2", target_bir_lowering=False, debug=False,
                   num_devices=N_CORES)
    TF = T * F
    pf_d = nc.dram_tensor("pf", [128, TF], dt.float32, kind="ExternalInput")
    basem_d = nc.dram_tensor("basem", [128, T], dt.float32, kind="ExternalInput")
    anw_d = nc.dram_tensor("anw", [128, T * 32], dt.int8, kind="ExternalInput")
    wts_d = nc.dram_tensor("wts", [128, T * 32], dt.float32, kind="ExternalInput")
    if phase == 2:
        mfac_d = nc.dram_tensor("mfac", [128, TF], dt.int32, kind="ExternalInput")
    mby_d = nc.dram_tensor("mby", [128, TF], dt.int8, kind="ExternalOutput")

    with TileContext(nc) as tc:
        with tc.tile_pool(name="big", bufs=1) as big, \
             tc.tile_pool(name="small", bufs=3) as small:
            pf = big.tile([128, TF], dt.float32)
            mby = big.tile([128, TF], dt.int8)
            basem = big.tile([128, T], dt.float32)
            anw = big.tile([128, T * 32], dt.int8)
            wts = big.tile([128, T * 32], dt.float32)
            nc.sync.dma_start(pf[:], pf_d.ap())
            nc.sync.dma_start(basem[:], basem_d.ap())
            nc.sync.dma_start(anw[:], anw_d.ap())
            nc.sync.dma_start(wts[:], wts_d.ap())
            if phase == 2:
                mfac = big.tile([128, TF], dt.int32)
                nc.sync.dma_start(mfac[:], mfac_d.ap())

            # ---- bitmap build: B[p, t] int32 of isSG over the chunk window
            anf = big.tile([128, T * 32], dt.float32)
            nc.vector.tensor_copy(anf[:], anw[:])
            eq = big.tile([128, T * 32], dt.float32)
            nc.vector.tensor_scalar(eq[:], anf[:], float(SG_HASH), None,
                                    AluOp.is_equal)
            nc.vector.tensor_tensor(eq[:], eq[:], wts[:], AluOp.mult)
            eq3 = eq[:].rearrange("p (t k) -> p t k", k=32)
            lo = big.tile([128, T], dt.float32)
            hi = big.tile([128, T], dt.float32)
            nc.vector.tensor_reduce(lo[:], eq3[:, :, 0:16],
                                    axis=mybir.AxisListType.X, op=AluOp.add)
            nc.vector.tensor_reduce(hi[:], eq3[:, :, 16:32],
                                    axis=mybir.AxisListType.X, op=AluOp.add)
            loi = big.tile([128, T], dt.int32)
            hii = big.tile([128, T], dt.int32)
            nc.vector.tensor_copy(loi[:], lo[:])
            nc.vector.tensor_copy(hii[:], hi[:])
            nc.vector.tensor_scalar(hii[:], hii[:], 16, None,
                                    AluOp.logical_shift_left)
            B = big.tile([128, T], dt.int32)
            nc.vector.tensor_tensor(B[:], loi[:], hii[:], AluOp.bitwise_or)

            # ---- per-tile mask: 5 fused passes
            for t in range(T):
                sl = slice(t * F, (t + 1) * F)
                ei = small.tile([128, F], dt.int32, tag="ei")
                pw = small.tile([128, F], dt.int32, tag="pw")
                mr = small.tile([128, F], dt.int32, tag="mr")
                # (pf - (base-127)) cast to int on write
                nc.vector.tensor_scalar(ei[:], pf[:, sl], basem[:, t:t + 1],
                                        None, AluOp.subtract)
                nc.vector.tensor_scalar(ei[:], ei[:], 23, None,
                                        AluOp.logical_shift_left)
                nc.vector.tensor_copy(pw[:], ei[:].bitcast(dt.float32))
                if phase == 2:
                    # (2^w & bitmap) & m1x  (m1x is 0 or ~0)
                    nc.vector.scalar_tensor_tensor(
                        mr[:], pw[:], B[:, t:t + 1], mfac[:, sl],
                        AluOp.bitwise_and, AluOp.bitwise_and)
                else:
                    nc.vector.tensor_scalar(mr[:], pw[:], B[:, t:t + 1], None,
                                            AluOp.bitwise_and)
                nc.vector.tensor_scalar(mby[:, sl], mr[:], 0, None,
                                        AluOp.not_equal)
            nc.sync.dma_start(mby_d.ap(), mby[:])
    nc.compile()
    return nc

def _get_program(T, phase):
    key = (T, phase)
    if key not in _prog_cache:
        _prog_cache[key] = _build_program(T, phase)
    return _prog_cache[key]


def _run_phase(nc, pf, basem, anw, wts, mfac=None):
    from concourse import bass_utils
    in_maps = []
    for c in range(N_CORES):
        m = dict(pf=np.ascontiguousarray(pf[c].reshape(128, -1)),
                 basem=np.ascontiguousarray(basem[c]),
                 anw=np.ascontiguousarray(anw[c].reshape(128, -1)),
                 wts=wts)
        if mfac is not None:
            m["mfac"] = np.ascontiguousarray(mfac[c].reshape(128, -1))
        in_maps.append(m)
    res = bass_utils.run_bass_kernel_spmd(nc, in_maps,
                                          core_ids=list(range(N_CORES)))
    return res


C_PAIR = 8            # pair columns per core in the tail (capacity 1024 pairs)
REC_P = 3936          # 128*3936 = 503808 >= N_ATOMS
NBINS = N_BATCH * N_CHAIN * N_RES


def _build_tail_program():
    import concourse.bacc as bacc
    import concourse.mybir as mybir
    from concourse.tile import TileContext
    dt = mybir.dt
    AluOp = mybir.AluOpType
    AF = mybir.ActivationFunctionType
    from concourse.bass import IndirectOffsetOnAxis

    C = C_PAIR
    nc = bacc.Bacc("TRN2", target_bir_lowering=False, debug=False,
                   num_devices=N_CORES)
    co_d = nc.dram_tensor("co", [128, REC_P * 3], dt.float32, kind="ExternalInput")
    adf_d = nc.dram_tensor("adf", [128, REC_P * 3], dt.float32, kind="ExternalInput")
    alb_d = nc.dram_tensor("alb", [128, REC_P], dt.float32, kind="ExternalInput")
    p0i_d = nc.dram_tensor("p0i", [128, C], dt.int32, kind="ExternalInput")
    p1i_d = nc.dram_tensor("p1i", [128, C], dt.int32, kind="ExternalInput")
    p0f_d = nc.dram_tensor("p0f", [128, C], dt.float32, kind="ExternalInput")
    ma0_d = nc.dram_tensor("ma0", [128, C * 128], dt.float32, kind="ExternalInput")
    ma1_d = nc.dram_tensor("ma1", [128, C * 128], dt.float32, kind="ExternalInput")
    mr0_d = nc.dram_tensor("mr0", [128, C * 128], dt.float32, kind="ExternalInput")
    mr1_d = nc.dram_tensor("mr1", [128, C * 128], dt.float32, kind="ExternalInput")
    prow_d = nc.dram_tensor("prow", [128, 1], dt.float32, kind="ExternalInput")
    ae_d = nc.dram_tensor("ae", [N_ATOMS + 128, A], dt.float32, kind="ExternalOutput")
    re_d = nc.dram_tensor("re", [NBINS + 128, A], dt.float32, kind="ExternalOutput")
    dbg_d = nc.dram_tensor("dbg", [128, C * 24], dt.float32, kind="ExternalOutput")
    dbg2_d = nc.dram_tensor("dbg2", [1, 128], dt.float32, kind="ExternalOutput")
    rec_t = nc.dram_tensor("rectab", [128 * REC_P, 8], dt.float32, kind="Internal")

    with TileContext(nc) as tc:
        with tc.tile_pool(name="big", bufs=1) as big, \
             tc.tile_pool(name="sm", bufs=2) as sm, \
             tc.tile_pool(name="ps", bufs=2, space="PSUM") as ps:
            rec_dram = rec_t
            # ---- build fat record table [atom, 8] = x,y,z,resnum,flat,altbits,0,0
            NSL = 4
            SL = REC_P // NSL
            for s in range(NSL):
                co = sm.tile([128, SL, 3], dt.float32, tag="co")
                adf = sm.tile([128, SL, 3], dt.float32, tag="adf")
                alb = sm.tile([128, SL], dt.float32, tag="alb")
                rec = sm.tile([128, SL, 8], dt.float32, tag="rec")
                nc.sync.dma_start(co[:], co_d.ap().rearrange(
                    "p (n c) -> p n c", c=3)[:, s * SL:(s + 1) * SL, :])
                nc.sync.dma_start(adf[:], adf_d.ap().rearrange(
                    "p (n c) -> p n c", c=3)[:, s * SL:(s + 1) * SL, :])
                nc.sync.dma_start(alb[:], alb_d.ap()[:, s * SL:(s + 1) * SL])
                for k in range(3):
                    nc.vector.tensor_copy(rec[:, :, k], co[:, :, k])
                nc.vector.tensor_copy(rec[:, :, 3], adf[:, :, 2])
                # flat = b*200000 + c*50000 + r
                nc.vector.tensor_scalar(rec[:, :, 4], adf[:, :, 0], 200000.0,
                                        None, AluOp.mult)
                fl2 = sm.tile([128, SL], dt.float32, tag="fl2")
                nc.vector.tensor_scalar(fl2[:], adf[:, :, 1], 50000.0, None,
                                        AluOp.mult)
                nc.vector.tensor_tensor(rec[:, :, 4], rec[:, :, 4], fl2[:],
                                        AluOp.add)
                nc.vector.tensor_tensor(rec[:, :, 4], rec[:, :, 4],
                                        adf[:, :, 2], AluOp.add)
                nc.vector.tensor_copy(rec[:, :, 5], alb[:])
                nc.gpsimd.memset(rec[:, :, 6:8], 0.0)
                nc.sync.dma_start(
                    rec_dram.ap().rearrange("(p n) e -> p n e", p=128)
                    [:, s * SL:(s + 1) * SL, :], rec[:])

            dbg2 = big.tile([1, 128], dt.float32)
            nc.sync.dma_start(dbg2[:], rec_t.ap()[0:16, :].rearrange("r e -> () (r e)"))
            nc.sync.dma_start(dbg2_d.ap(), dbg2[:])
            p0i = big.tile([128, C], dt.int32)
            p1i = big.tile([128, C], dt.int32)
            p0f = big.tile([128, C], dt.float32)
            prow_d_col = big.tile([128, 1], dt.float32)
            nc.sync.dma_start(p0i[:], p0i_d.ap())
            nc.sync.dma_start(p1i[:], p1i_d.ap())
            nc.sync.dma_start(p0f[:], p0f_d.ap())
            nc.sync.dma_start(prow_d_col[:], prow_d.ap())
            rec0f = big.tile([128, C * 8], dt.float32)
            rec1f = big.tile([128, C * 8], dt.float32)
            nc.gpsimd.memset(rec0f[:], 0.0)
            nc.gpsimd.memset(rec1f[:], 0.0)
            for c in range(C):
                nc.gpsimd.indirect_dma_start(
                    rec0f[:, c * 8:(c + 1) * 8], None, rec_dram.ap(),
                    IndirectOffsetOnAxis(ap=p0i[:, c:c + 1], axis=0))
                nc.gpsimd.indirect_dma_start(
                    rec1f[:, c * 8:(c + 1) * 8], None, rec_dram.ap(),
                    IndirectOffsetOnAxis(ap=p1i[:, c:c + 1], axis=0))
            rec0 = rec0f[:].rearrange("p (c e) -> p c e", e=8)
            rec1 = rec1f[:].rearrange("p (c e) -> p c e", e=8)

            # ---- energy per pair slot [128, C]
            d2 = big.tile([128, C], dt.float32)
            tmp = big.tile([128, C], dt.float32)
            nc.gpsimd.memset(d2[:], 0.0)
            for k in range(3):
                dx = sm.tile([128, C], dt.float32, tag="dx")
                nc.vector.tensor_tensor(dx[:], rec0[:, :, k], rec1[:, :, k],
                                        AluOp.subtract)
                nc.vector.tensor_scalar(dx[:], dx[:], 1e-6, None, AluOp.add)
                nc.vector.tensor_tensor(dx[:], dx[:], dx[:], AluOp.mult)
                nc.vector.tensor_tensor(d2[:], d2[:], dx[:], AluOp.add)
            dist = big.tile([128, C], dt.float32)
            nc.scalar.activation(dist[:], d2[:], mybir.ActivationFunctionType.Sqrt)
            padv01 = big.tile([128, C], dt.float32)
            nc.vector.tensor_scalar(padv01[:], p0f[:], float(N_ATOMS) - 0.5,
                                    None, AluOp.is_gt)
            rd = big.tile([128, C], dt.float32)
            nc.vector.tensor_tensor(rd[:], rec0[:, :, 3], rec1[:, :, 3],
                                    AluOp.subtract)
            nc.scalar.activation(rd[:], rd[:], mybir.ActivationFunctionType.Abs)
            nc.vector.tensor_tensor(rd[:], rd[:], padv01[:], AluOp.add)
            lg = big.tile([128, C], dt.float32)
            nc.scalar.activation(lg[:], rd[:], mybir.ActivationFunctionType.Ln)
            netE = big.tile([128, C], dt.float32)
            # netE = 0.5*(-0.298*(2.1 + 2.9823825*lg) + 5*|dist-2.04|)
            nc.vector.tensor_scalar(netE[:], lg[:],
                                    0.5 * -0.001 * TEMPERATURE * 2.9823825,
                                    0.5 * -0.001 * TEMPERATURE * 2.1,
                                    AluOp.mult, AluOp.add)
            nc.vector.tensor_scalar(tmp[:], dist[:], 2.04, None,
                                    AluOp.subtract)
            nc.scalar.activation(tmp[:], tmp[:], mybir.ActivationFunctionType.Abs)
            nc.vector.tensor_scalar(tmp[:], tmp[:], 2.5, None, AluOp.mult)
            nc.vector.tensor_tensor(netE[:], netE[:], tmp[:], AluOp.add)
            # alt bits -> contrib [128, C, 4]
            ab = big.tile([128, C], dt.int32)
            a0i = big.tile([128, C], dt.int32)
            a1i = big.tile([128, C], dt.int32)
            nc.vector.tensor_copy(a0i[:], rec0[:, :, 5])
            nc.vector.tensor_copy(a1i[:], rec1[:, :, 5])
            nc.vector.tensor_tensor(ab[:], a0i[:], a1i[:], AluOp.bitwise_and)
            contrib = big.tile([128, C, 4], dt.float32)
            for k in range(A):
                bk = sm.tile([128, C], dt.int32, tag="bk")
                bf = sm.tile([128, C], dt.float32, tag="bf")
                nc.vector.tensor_scalar(bk[:], ab[:], 1 << k, None,
                                        AluOp.bitwise_and)
                nc.vector.tensor_copy(bf[:], bk[:])
                nc.vector.tensor_scalar(bf[:], bf[:], 0.0, None, AluOp.not_equal)
                nc.vector.tensor_tensor(contrib[:, :, k], netE[:], bf[:],
                                        AluOp.mult)

            # ---- flat offsets (+pad fix: pads have p0f > N_ATOMS)
            # pads write to dump rows NBINS+p (sliced off on host)
            prow = big.tile([128, C], dt.float32)
            nc.vector.tensor_scalar(prow[:], padv01[:], prow_d_col[:], None,
                                    AluOp.mult)
            vm = big.tile([128, C], dt.float32)
            nc.vector.tensor_scalar(vm[:], padv01[:], -1.0, 1.0, AluOp.mult,
                                    AluOp.add)
            f0 = big.tile([128, C], dt.float32)
            f1 = big.tile([128, C], dt.float32)
            nc.vector.tensor_tensor(f0[:], rec0[:, :, 4], vm[:], AluOp.mult)
            nc.vector.tensor_tensor(f0[:], f0[:], prow[:], AluOp.add)
            nc.vector.tensor_tensor(f1[:], rec1[:, :, 4], vm[:], AluOp.mult)
            nc.vector.tensor_tensor(f1[:], f1[:], prow[:], AluOp.add)
            f0i = big.tile([128, C], dt.int32)
            f1i = big.tile([128, C], dt.int32)
            nc.vector.tensor_copy(f0i[:], f0[:])
            nc.vector.tensor_copy(f1i[:], f1[:])

            # ---- dedup totals via per-column group matmuls, then write scatters
            ma0 = big.tile([128, C * 128], dt.float32)
            ma1 = big.tile([128, C * 128], dt.float32)
            mr0 = big.tile([128, C * 128], dt.float32)
            mr1 = big.tile([128, C * 128], dt.float32)
            nc.sync.dma_start(ma0[:], ma0_d.ap())
            nc.sync.dma_start(ma1[:], ma1_d.ap())
            nc.sync.dma_start(mr0[:], mr0_d.ap())
            nc.sync.dma_start(mr1[:], mr1_d.ap())
            dbg = big.tile([128, C * 24], dt.float32)
            nc.vector.tensor_copy(dbg[:].rearrange("p (c e) -> p c e", e=24)[:, :, 0:8], rec0)
            nc.vector.tensor_copy(dbg[:].rearrange("p (c e) -> p c e", e=24)[:, :, 8:16], rec1)
            nc.vector.tensor_copy(dbg[:].rearrange("p (c e) -> p c e", e=24)[:, :, 16:17], netE[:].rearrange("p c -> p c ()"))
            nc.vector.tensor_copy(dbg[:].rearrange("p (c e) -> p c e", e=24)[:, :, 17:21], contrib[:])
            nc.vector.tensor_copy(dbg[:].rearrange("p (c e) -> p c e", e=24)[:, :, 21:22], rd[:].rearrange("p c -> p c ()"))
            nc.vector.tensor_copy(dbg[:].rearrange("p (c e) -> p c e", e=24)[:, :, 22:23], dist[:].rearrange("p c -> p c ()"))
            nc.sync.dma_start(dbg_d.ap(), dbg[:])
            tots = {}
            for name, m in (("a0", ma0), ("a1", ma1), ("r0", mr0), ("r1", mr1)):
                tot = big.tile([128, C * 4], dt.float32, tag="tot" + name)
                for c in range(C):
                    pt = ps.tile([128, 4], dt.float32, tag="pt")
                    nc.tensor.matmul(out=pt[:], lhsT=m[:, c * 128:(c + 1) * 128],
                                     rhs=contrib[:, c, :], start=True, stop=True)
                    nc.vector.tensor_copy(tot[:, c * 4:(c + 1) * 4], pt[:])
                tots[name] = tot
            for c in range(C):
                nc.gpsimd.indirect_dma_start(
                    ae_d.ap(), IndirectOffsetOnAxis(ap=p0i[:, c:c + 1], axis=0),
                    tots["a0"][:, c * 4:(c + 1) * 4], None)
                nc.gpsimd.indirect_dma_start(
                    ae_d.ap(), IndirectOffsetOnAxis(ap=p1i[:, c:c + 1], axis=0),
                    tots["a1"][:, c * 4:(c + 1) * 4], None)
                nc.gpsimd.indirect_dma_start(
                    re_d.ap(), IndirectOffsetOnAxis(ap=f0i[:, c:c + 1], axis=0),
                    tots["r0"][:, c * 4:(c + 1) * 4], None)
                nc.gpsimd.indirect_dma_start(
                    re_d.ap(), IndirectOffsetOnAxis(ap=f1i[:, c:c + 1], axis=0),
                    tots["r1"][:, c * 4:(c + 1) * 4], None)
    nc.compile()
    return nc


def _get_tail_program():
    if "tail" not in _prog_cache:
        _prog_cache["tail"] = _build_tail_program()
    return _prog_cache["tail"]


class _UF:
    def __init__(self, n):
        self.p = list(range(n))

    def find(self, x):
        while self.p[x] != x:
            self.p[x] = self.p[self.p[x]]
            x = self.p[x]
        return x

    def union(self, a, b):
        ra, rb = self.find(a), self.find(b)
        if ra != rb:
            self.p[ra] = rb


def _pack_tail_core(h0, h1, fl0, fl1):
    """Pack this core's hit pairs into [128, C_PAIR] columns such that no two
    columns share an atom id or flat id. Returns (rows, cols, ok)."""
    n = len(h0)
    C = C_PAIR
    if n == 0:
        return np.zeros(0, np.int64), np.zeros(0, np.int64), True
    uf = _UF(n)
    for keys in (np.concatenate([h0, h1]), np.concatenate([fl0, fl1])):
        pid = np.tile(np.arange(n), 2)
        o = np.argsort(keys, kind="stable")
        ks, po = keys[o], pid[o]
        same = np.flatnonzero(ks[1:] == ks[:-1])
        for i in same:
            uf.union(int(po[i]), int(po[i + 1]))
    root = np.array([uf.find(i) for i in range(n)])
    comps = {}
    for i, r in enumerate(root):
        comps.setdefault(r, []).append(i)
    # first-fit into C columns of capacity 128
    cap = [128] * C
    rows = np.empty(n, np.int64)
    cols = np.empty(n, np.int64)
    for members in sorted(comps.values(), key=len, reverse=True):
        sz = len(members)
        if sz > 128:
            return rows, cols, False
        placed = False
        for c in range(C):
            if cap[c] >= sz:
                r0 = 128 - cap[c]
                for j, m in enumerate(members):
                    rows[m] = r0 + j
                    cols[m] = c
                cap[c] -= sz
                placed = True
                break
        if not placed:
            return rows, cols, False
    return rows, cols, True


def _host_tail(coords, resnum, alt, p0, p1, hit):
    atomEnergy = np.zeros((N_ATOMS, A), np.float32)
    if len(hit):
        h0, h1 = p0[hit], p1[hit]
        diff = coords[h0] - coords[h1] + 1e-6
        dist = np.sqrt(np.sum(diff * diff, axis=-1))
        rd = np.abs(resnum[h0] - resnum[h1]).astype(np.float32)
        energy = -0.001 * TEMPERATURE * (2.1 + 2.9823825 * np.log(rd)) \
            + 5.0 * np.abs(dist - 2.04)
        netE = 0.5 * energy
        pair_alt = alt[h0] & alt[h1]
        contrib = np.where(pair_alt, netE[:, None], 0.0).astype(np.float32)
        np.add.at(atomEnergy, h0, contrib)
        np.add.at(atomEnergy, h1, contrib)
    return atomEnergy


def kernel(coords, atom_description, atom_number, atomPairs, alternativeMask,
           partners, facc):
    coords = np.asarray(coords, np.float32)
    ad = np.asarray(atom_description)
    pairs = np.asarray(atomPairs).astype(np.int64)
    alt = np.asarray(alternativeMask).astype(bool)
    at_name = ad[:, 3].astype(np.int64)
    resnum = ad[:, 2].astype(np.int64)
    p0, p1 = pairs[:, 0], pairs[:, 1]
    npair = len(p0)

    at_u8 = at_name.astype(np.uint8)

    # choose T adaptively (worst-case chunk count over both sides)
    def n_chunks_needed(p):
        win_counts = np.bincount(p // WIN)
        wc = win_counts[win_counts > 0]
        return int(np.sum((wc + F - 1) // F))

    need = max(n_chunks_needed(p0), n_chunks_needed(p1))
    T = max(1, -(-need // (8 * 128)))
    nc1 = _get_program(T, 1)
    nc2 = _get_program(T, 2)

    # weights for bitmap packing
    k = np.arange(32)
    w_lo = np.where(k < 16, 2.0 ** k, 0.0)
    w_hi = np.where(k >= 16, 2.0 ** (k - 16), 0.0)
    wts = np.tile((w_lo + w_hi).astype(np.float32), T)[None, :].repeat(128, 0)
    wts[:, 31::32] = 0.0

    # ---- phase 1: p1 side
    slot1, pf1, basem1, boc1, _ = _build_layout(p1, T)
    anw1 = _build_anw(at_u8, boc1)
    res1 = _run_phase(nc1, pf1, basem1, anw1, wts)
    m1 = np.stack([res1.results[c]["mby"] for c in range(N_CORES)])  # [8,128,TF]
    m1_of_pair = m1.reshape(-1)[slot1]

    # ---- relay into p0 layout (as 0 / ~0 int32 so phase 2 is all-bitwise)
    slot0, pf0, basem0, boc0, _ = _build_layout(p0, T)
    anw0 = _build_anw(at_u8, boc0)
    mfac0 = np.zeros(8 * 128 * T * F, np.int32)
    mfac0[slot0] = -m1_of_pair.astype(np.int32)
    res0 = _run_phase(nc2, pf0, basem0, anw0, wts,
                      mfac0.reshape(8, 128, T * F))
    sby = np.stack([res0.results[c]["mby"] for c in range(N_CORES)])
    sulfur = sby.reshape(-1)[slot0] != 0

    # ---- sparse tail on device (phase 3); host fallback for over-capacity
    flat = ((ad[:, 0].astype(np.int64) * N_CHAIN + ad[:, 1]) * N_RES + resnum)
    hit = np.flatnonzero(sulfur)
    # Device tail by default; host fallback only for over-capacity inputs
    # (>128*C_PAIR sulfur pairs per core) or if explicitly disabled.
    import os
    use_device_tail = (os.environ.get("DISULF_DEVICE_TAIL", "1") == "1"
                       and len(hit) <= N_CORES * 128 * C_PAIR)

    if use_device_tail:
        from concourse import bass_utils
        nct = _get_tail_program()
        C = C_PAIR
        # shared per-atom inputs, padded to 128*REC_P
        NP_ = 128 * REC_P
        co_pad = np.zeros((NP_, 3), np.float32)
        co_pad[:N_ATOMS] = coords
        adf_pad = np.zeros((NP_, 3), np.float32)
        adf_pad[:N_ATOMS, 0] = ad[:, 0]
        adf_pad[:N_ATOMS, 1] = ad[:, 1]
        adf_pad[:N_ATOMS, 2] = resnum
        alb_pad = np.zeros(NP_, np.float32)
        alb_pad[:N_ATOMS] = (alt * (1 << np.arange(4))).sum(1)
        co_in = co_pad.reshape(128, REC_P * 3)
        adf_in = adf_pad.reshape(128, REC_P * 3)
        alb_in = alb_pad.reshape(128, REC_P)
        # split hits evenly across cores
        in_maps = []
        per = [hit[c::N_CORES] for c in range(N_CORES)]
        ok_all = True
        for c in range(N_CORES):
            hc = per[c]
            h0, h1 = p0[hc], p1[hc]
            f0, f1 = flat[h0], flat[h1]
            rows, cols, ok = _pack_tail_core(h0, h1, f0, f1)
            ok_all &= ok
            if not ok:
                break
            rowpad = (N_ATOMS + np.arange(128, dtype=np.int32))[:, None]
            p0i = np.broadcast_to(rowpad, (128, C)).astype(np.int32).copy()
            p1i = p0i.copy()
            # pad ids N_ATOMS+p: never equal a real atom; within a column all distinct
            a0 = p0i.astype(np.int64).copy()
            a1 = a0.copy()
            g0 = np.broadcast_to(NBINS + np.arange(128)[:, None],
                                 (128, C)).astype(np.int64).copy()
            g1 = g0.copy()
            a0[rows, cols] = h0
            a1[rows, cols] = h1
            g0[rows, cols] = f0
            g1[rows, cols] = f1
            p0i[rows, cols] = h0
            p1i[rows, cols] = h1
            ma0 = np.zeros((128, C, 128), np.float32)
            ma1 = np.zeros((128, C, 128), np.float32)
            mr0 = np.zeros((128, C, 128), np.float32)
            mr1 = np.zeros((128, C, 128), np.float32)
            for cc in range(C):
                A0, A1 = a0[:, cc], a1[:, cc]
                G0, G1 = g0[:, cc], g1[:, cc]
                ma0[:, cc, :] = ((A0[:, None] == A0[None, :]) +
                                 (A1[:, None] == A0[None, :])).astype(np.float32)
                ma1[:, cc, :] = ((A0[:, None] == A1[None, :]) +
                                 (A1[:, None] == A1[None, :])).astype(np.float32)
                mr0[:, cc, :] = ((G0[:, None] == G0[None, :]) +
                                 (G1[:, None] == G0[None, :])).astype(np.float32)
                mr1[:, cc, :] = ((G0[:, None] == G1[None, :]) +
                                 (G1[:, None] == G1[None, :])).astype(np.float32)
            in_maps.append(dict(
                co=co_in, adf=adf_in, alb=alb_in,
                p0i=p0i, p1i=p1i, p0f=p0i.astype(np.float32),
                ma0=ma0.reshape(128, -1), ma1=ma1.reshape(128, -1),
                mr0=mr0.reshape(128, -1), mr1=mr1.reshape(128, -1),
                prow=(NBINS + np.arange(128, dtype=np.float32))[:, None]))
        if ok_all:
            rest = bass_utils.run_bass_kernel_spmd(
                nct, in_maps, core_ids=list(range(N_CORES)))
            atomEnergy = np.zeros((N_ATOMS, A), np.float32)
            residueEnergy = np.zeros((NBINS, A), np.float32)
            for c in range(N_CORES):
                atomEnergy += rest.results[c]["ae"][:N_ATOMS]
                residueEnergy += rest.results[c]["re"][:NBINS]
        else:
            use_device_tail = False

    if not use_device_tail:
        atomEnergy = _host_tail(coords, resnum, alt, p0, p1, hit)
        residueEnergy = np.zeros((NBINS, A), np.float32)
        nz = np.flatnonzero(np.any(atomEnergy != 0, axis=1))
        np.add.at(residueEnergy, flat[nz], atomEnergy[nz])

    residueEnergy = residueEnergy.reshape(N_BATCH, N_CHAIN, N_RES, A)
    return residueEnergy, atomEnergy, sulfur


# revision 18
# speedup vs baseline: 1.5455x; 1.1515x over previous
"""Trainium2 Bass kernel for nn_Disulfide_net (edge-parallel GNN scatter).

Strategy (8 NeuronCores, SPMD):
  - Host shards atomPairs by sorting each endpoint column and grouping pairs
    into "chunks" of <=F pairs whose endpoint atoms span a <=31-atom window
    (graph-partition sharding + per-chunk halo of at_name bytes).
  - Device phase 1 (p1-sorted layout): per chunk, pack isSG (at_name==16) of
    the 31-atom window into an int32 bitmap held as a per-partition scalar;
    each pair tests its bit via an exponent-trick (1<<w built from float bits)
    -> m1 mask for all 8M pairs at ~1/128 cycle each.
  - Host relays m1 into the p0-sorted layout (pure index permutation).
  - Device phase 2 (p0-sorted layout): m0 bit-test AND m1 -> sulfur mask.
  - Device phase 3 (sparse tail, ~1/1024 of pairs): indirect-DMA gathers of
    per-atom records, dense energy evaluation, duplicate-group totals via
    per-column PE matmuls (host supplies 0/1 group matrices from pure index
    data), then indirect-DMA row writes into per-core atomEnergy /
    residueEnergy partials; host sums the 8 partials on unshard.
"""
import sys
import numpy as np

sys.path.insert(0, "/opt/trn_rl_repo")

N_CORES = 8
N_ATOMS = 500_000
N_PAIRS = 8_000_000
A = 4
N_BATCH, N_CHAIN, N_RES = 2, 4, 50000
SG_HASH = 16
TEMPERATURE = 298.0
WIN = 31          # atoms per chunk window (bit 31 of int32 never used)
F = 576           # pair slots per chunk (~16% pad at lambda~496)
_prog_cache = {}


def _build_layout(pvals, T):
    """Assign each pair a slot in the [8, 128, T, F] chunked layout, sorted by
    its endpoint atom id. Returns (slot_of_pair, per-core arrays)."""
    order = np.argsort(pvals, kind="stable")
    ps = pvals[order]
    win = ps // WIN
    uw, first = np.unique(win, return_index=True)
    cnts = np.diff(np.r_[first, len(ps)])
    nchunk_per_win = (cnts + F - 1) // F
    chunk_base = np.r_[0, np.cumsum(nchunk_per_win)][:-1]
    n_chunks = int(chunk_base[-1] + nchunk_per_win[-1]) if len(uw) else 0
    assert n_chunks <= 8 * 128 * T, f"{n_chunks} chunks > capacity {8*128*T}"
    rank = np.arange(len(ps)) - np.repeat(first, cnts)
    chunk_of = np.repeat(chunk_base, cnts) + rank // F
    slot_in = rank % F
    # chunk c -> core = c // (128*T), p = (c % (128*T)) % 128, t = (...) // 128
    c_loc = chunk_of % (128 * T)
    # [core, p, t, f] flat = ((core*128 + p)*T + t)*F + f
    slot = ((chunk_of // (128 * T)) * 128 + (c_loc % 128)) * (T * F) \
        + (c_loc // 128) * F + slot_in
    slot_of_pair = np.empty(len(ps), np.int64)
    slot_of_pair[order] = slot
    # per-chunk metadata, scattered into the [core, p, t] layout order
    cid = np.arange(n_chunks)
    cl = cid % (128 * T)
    lay = ((cid // (128 * T)) * 128 + (cl % 128)) * T + (cl // 128)
    base_of_chunk = np.zeros(8 * 128 * T, np.int64)
    base_of_chunk[lay] = np.repeat(uw, nchunk_per_win) * WIN
    pf = np.zeros((8, 128, T, F), np.float32)
    pf.reshape(-1)[slot] = ps  # pad slots keep 0 -> w may alias bit0; killed by mfac
    # pads must stay in-window: set pad slots' value to the chunk's base
    padmask = np.ones(8 * 128 * T * F, bool)
    padmask[slot] = False
    cb = np.repeat(base_of_chunk, F)
    pfflat = pf.reshape(-1)
    pfflat[padmask] = cb[padmask]
    basem = (base_of_chunk.astype(np.float32) - 127.0)
    basem = basem.reshape(8, 128, T)
    return slot_of_pair, pf, basem, base_of_chunk.reshape(8, 128, T), n_chunks


def _build_anw(at_name_u8, base_of_chunk):
    """Per-chunk 32-byte at_name halo windows. [8,128,T,32] int8."""
    idx = base_of_chunk[..., None] + np.arange(32)
    valid = idx < N_ATOMS
    idxc = np.clip(idx, 0, N_ATOMS - 1)
    anw = at_name_u8[idxc]
    anw[~valid] = 0
    anw[..., 31] = 0  # bit31 unused
    return anw.astype(np.int8)


def _build_program(T, phase):
    import concourse.bacc as bacc
    import concourse.mybir as mybir
    from concourse.tile import TileContext
    dt = mybir.dt
    AluOp = mybir.AluOpType

    nc = bacc.Bacc("TRN2", target_bir_lowering=False, debug=False,
                   num_devices=N_CORES)
    TF = T * F
    pf_d = nc.dram_tensor("pf", [128, TF], dt.float32, kind="ExternalInput")
    basem_d = nc.dram_tensor("basem", [128, T], dt.float32, kind="ExternalInput")
    anw_d = nc.dram_tensor("anw", [128, T * 32], dt.int8, kind="ExternalInput")
    wts_d = nc.dram_tensor("wts", [128, T * 32], dt.float32, kind="ExternalInput")
    if phase == 2:
        mfac_d = nc.dram_tensor("mfac", [128, TF], dt.int8, kind="ExternalInput")
    mby_d = nc.dram_tensor("mby", [128, TF], dt.int8, kind="ExternalOutput")

    with TileContext(nc) as tc:
        with tc.tile_pool(name="big", bufs=1) as big, \
             tc.tile_pool(name="small", bufs=3) as small:
            pf = big.tile([128, TF], dt.float32)
            mby = big.tile([128, TF], dt.int8)
            basem = big.tile([128, T], dt.float32)
            anw = big.tile([128, T * 32], dt.int8)
            wts = big.tile([128, T * 32], dt.float32)
            nc.sync.dma_start(pf[:], pf_d.ap())
            nc.sync.dma_start(basem[:], basem_d.ap())
            nc.sync.dma_start(anw[:], anw_d.ap())
            nc.sync.dma_start(wts[:], wts_d.ap())
            if phase == 2:
                mfac = big.tile([128, TF], dt.int8)
                nc.sync.dma_start(mfac[:], mfac_d.ap())

            # ---- bitmap build: B[p, t] int32 of isSG over the chunk window
            anf = big.tile([128, T * 32], dt.float32)
            nc.vector.tensor_copy(anf[:], anw[:])
            eq = big.tile([128, T * 32], dt.float32)
            nc.vector.tensor_scalar(eq[:], anf[:], float(SG_HASH), None,
                                    AluOp.is_equal)
            nc.vector.tensor_tensor(eq[:], eq[:], wts[:], AluOp.mult)
            eq3 = eq[:].rearrange("p (t k) -> p t k", k=32)
            lo = big.tile([128, T], dt.float32)
            hi = big.tile([128, T], dt.float32)
            nc.vector.tensor_reduce(lo[:], eq3[:, :, 0:16],
                                    axis=mybir.AxisListType.X, op=AluOp.add)
            nc.vector.tensor_reduce(hi[:], eq3[:, :, 16:32],
                                    axis=mybir.AxisListType.X, op=AluOp.add)
            loi = big.tile([128, T], dt.int32)
            hii = big.tile([128, T], dt.int32)
            nc.vector.tensor_copy(loi[:], lo[:])
            nc.vector.tensor_copy(hii[:], hi[:])
            nc.vector.tensor_scalar(hii[:], hii[:], 16, None,
                                    AluOp.logical_shift_left)
            B = big.tile([128, T], dt.int32)
            nc.vector.tensor_tensor(B[:], loi[:], hii[:], AluOp.bitwise_or)

            # ---- per-tile mask: 5 fused passes
            for t in range(T):
                sl = slice(t * F, (t + 1) * F)
                ei = small.tile([128, F], dt.int32, tag="ei")
                pw = small.tile([128, F], dt.int32, tag="pw")
                mr = small.tile([128, F], dt.int32, tag="mr")
                # (pf - (base-127)) cast to int on write
                nc.vector.tensor_scalar(ei[:], pf[:, sl], basem[:, t:t + 1],
                                        None, AluOp.subtract)
                nc.vector.tensor_scalar(ei[:], ei[:], 23, None,
                                        AluOp.logical_shift_left)
                nc.vector.tensor_copy(pw[:], ei[:].bitcast(dt.float32))
                nc.vector.tensor_scalar(mr[:], pw[:], B[:, t:t + 1], None,
                                        AluOp.bitwise_and)
                if phase == 2:
                    # (bit != 0) AND m1 {0,1} int8 relay
                    nc.vector.scalar_tensor_tensor(
                        mby[:, sl], mr[:], 0, mfac[:, sl],
                        AluOp.not_equal, AluOp.logical_and)
                else:
                    nc.vector.tensor_scalar(mby[:, sl], mr[:], 0, None,
                                            AluOp.not_equal)
            nc.sync.dma_start(mby_d.ap(), mby[:])
    nc.compile()
    return nc

def _get_program(T, phase):
    key = (T, phase)
    if key not in _prog_cache:
        _prog_cache[key] = _build_program(T, phase)
    return _prog_cache[key]


def _run_phase(nc, pf, basem, anw, wts, mfac=None):
    from concourse import bass_utils
    in_maps = []
    for c in range(N_CORES):
        m = dict(pf=np.ascontiguousarray(pf[c].reshape(128, -1)),
                 basem=np.ascontiguousarray(basem[c]),
                 anw=np.ascontiguousarray(anw[c].reshape(128, -1)),
                 wts=wts)
        if mfac is not None:
            m["mfac"] = np.ascontiguousarray(mfac[c].reshape(128, -1))
        in_maps.append(m)
    res = bass_utils.run_bass_kernel_spmd(nc, in_maps,
                                          core_ids=list(range(N_CORES)))
    return res


C_PAIR = 8            # pair columns per core in the tail (capacity 1024 pairs)
REC_P = 3936          # 128*3936 = 503808 >= N_ATOMS
NBINS = N_BATCH * N_CHAIN * N_RES


def _build_tail_program():
    import concourse.bacc as bacc
    import concourse.mybir as mybir
    from concourse.tile import TileContext
    dt = mybir.dt
    AluOp = mybir.AluOpType
    AF = mybir.ActivationFunctionType
    from concourse.bass import IndirectOffsetOnAxis

    C = C_PAIR
    nc = bacc.Bacc("TRN2", target_bir_lowering=False, debug=False,
                   num_devices=N_CORES)
    co_d = nc.dram_tensor("co", [128, REC_P * 3], dt.float32, kind="ExternalInput")
    adf_d = nc.dram_tensor("adf", [128, REC_P * 3], dt.float32, kind="ExternalInput")
    alb_d = nc.dram_tensor("alb", [128, REC_P], dt.float32, kind="ExternalInput")
    p0i_d = nc.dram_tensor("p0i", [128, C], dt.int32, kind="ExternalInput")
    p1i_d = nc.dram_tensor("p1i", [128, C], dt.int32, kind="ExternalInput")
    p0f_d = nc.dram_tensor("p0f", [128, C], dt.float32, kind="ExternalInput")
    ma0_d = nc.dram_tensor("ma0", [128, C * 128], dt.float32, kind="ExternalInput")
    ma1_d = nc.dram_tensor("ma1", [128, C * 128], dt.float32, kind="ExternalInput")
    mr0_d = nc.dram_tensor("mr0", [128, C * 128], dt.float32, kind="ExternalInput")
    mr1_d = nc.dram_tensor("mr1", [128, C * 128], dt.float32, kind="ExternalInput")
    prow_d = nc.dram_tensor("prow", [128, 1], dt.float32, kind="ExternalInput")
    ae_d = nc.dram_tensor("ae", [N_ATOMS + 128, A], dt.float32, kind="ExternalOutput")
    re_d = nc.dram_tensor("re", [NBINS + 128, A], dt.float32, kind="ExternalOutput")
    dbg_d = nc.dram_tensor("dbg", [128, C * 24], dt.float32, kind="ExternalOutput")
    dbg2_d = nc.dram_tensor("dbg2", [1, 128], dt.float32, kind="ExternalOutput")
    rec_t = nc.dram_tensor("rectab", [128 * REC_P, 8], dt.float32, kind="Internal")

    with TileContext(nc) as tc:
        with tc.tile_pool(name="big", bufs=1) as big, \
             tc.tile_pool(name="sm", bufs=2) as sm, \
             tc.tile_pool(name="ps", bufs=2, space="PSUM") as ps:
            rec_dram = rec_t
            # ---- build fat record table [atom, 8] = x,y,z,resnum,flat,altbits,0,0
            NSL = 4
            SL = REC_P // NSL
            for s in range(NSL):
                co = sm.tile([128, SL, 3], dt.float32, tag="co")
                adf = sm.tile([128, SL, 3], dt.float32, tag="adf")
                alb = sm.tile([128, SL], dt.float32, tag="alb")
                rec = sm.tile([128, SL, 8], dt.float32, tag="rec")
                nc.sync.dma_start(co[:], co_d.ap().rearrange(
                    "p (n c) -> p n c", c=3)[:, s * SL:(s + 1) * SL, :])
                nc.sync.dma_start(adf[:], adf_d.ap().rearrange(
                    "p (n c) -> p n c", c=3)[:, s * SL:(s + 1) * SL, :])
                nc.sync.dma_start(alb[:], alb_d.ap()[:, s * SL:(s + 1) * SL])
                for k in range(3):
                    nc.vector.tensor_copy(rec[:, :, k], co[:, :, k])
                nc.vector.tensor_copy(rec[:, :, 3], adf[:, :, 2])
                # flat = b*200000 + c*50000 + r
                nc.vector.tensor_scalar(rec[:, :, 4], adf[:, :, 0], 200000.0,
                                        None, AluOp.mult)
                fl2 = sm.tile([128, SL], dt.float32, tag="fl2")
                nc.vector.tensor_scalar(fl2[:], adf[:, :, 1], 50000.0, None,
                                        AluOp.mult)
                nc.vector.tensor_tensor(rec[:, :, 4], rec[:, :, 4], fl2[:],
                                        AluOp.add)
                nc.vector.tensor_tensor(rec[:, :, 4], rec[:, :, 4],
                                        adf[:, :, 2], AluOp.add)
                nc.vector.tensor_copy(rec[:, :, 5], alb[:])
                nc.gpsimd.memset(rec[:, :, 6:8], 0.0)
                nc.sync.dma_start(
                    rec_dram.ap().rearrange("(p n) e -> p n e", p=128)
                    [:, s * SL:(s + 1) * SL, :], rec[:])

            dbg2 = big.tile([1, 128], dt.float32)
            nc.sync.dma_start(dbg2[:], rec_t.ap()[0:16, :].rearrange("r e -> () (r e)"))
            nc.sync.dma_start(dbg2_d.ap(), dbg2[:])
            p0i = big.tile([128, C], dt.int32)
            p1i = big.tile([128, C], dt.int32)
            p0f = big.tile([128, C], dt.float32)
            prow_d_col = big.tile([128, 1], dt.float32)
            nc.sync.dma_start(p0i[:], p0i_d.ap())
            nc.sync.dma_start(p1i[:], p1i_d.ap())
            nc.sync.dma_start(p0f[:], p0f_d.ap())
            nc.sync.dma_start(prow_d_col[:], prow_d.ap())
            rec0f = big.tile([128, C * 8], dt.float32)
            rec1f = big.tile([128, C * 8], dt.float32)
            nc.gpsimd.memset(rec0f[:], 0.0)
            nc.gpsimd.memset(rec1f[:], 0.0)
            for c in range(C):
                nc.gpsimd.indirect_dma_start(
                    rec0f[:, c * 8:(c + 1) * 8], None, rec_dram.ap(),
                    IndirectOffsetOnAxis(ap=p0i[:, c:c + 1], axis=0))
                nc.gpsimd.indirect_dma_start(
                    rec1f[:, c * 8:(c + 1) * 8], None, rec_dram.ap(),
                    IndirectOffsetOnAxis(ap=p1i[:, c:c + 1], axis=0))
            rec0 = rec0f[:].rearrange("p (c e) -> p c e", e=8)
            rec1 = rec1f[:].rearrange("p (c e) -> p c e", e=8)

            # ---- energy per pair slot [128, C]
            d2 = big.tile([128, C], dt.float32)
            tmp = big.tile([128, C], dt.float32)
            nc.gpsimd.memset(d2[:], 0.0)
            for k in range(3):
                dx = sm.tile([128, C], dt.float32, tag="dx")
                nc.vector.tensor_tensor(dx[:], rec0[:, :, k], rec1[:, :, k],
                                        AluOp.subtract)
                nc.vector.tensor_scalar(dx[:], dx[:], 1e-6, None, AluOp.add)
                nc.vector.tensor_tensor(dx[:], dx[:], dx[:], AluOp.mult)
                nc.vector.tensor_tensor(d2[:], d2[:], dx[:], AluOp.add)
            dist = big.tile([128, C], dt.float32)
            nc.scalar.activation(dist[:], d2[:], mybir.ActivationFunctionType.Sqrt)
            padv01 = big.tile([128, C], dt.float32)
            nc.vector.tensor_scalar(padv01[:], p0f[:], float(N_ATOMS) - 0.5,
                                    None, AluOp.is_gt)
            rd = big.tile([128, C], dt.float32)
            nc.vector.tensor_tensor(rd[:], rec0[:, :, 3], rec1[:, :, 3],
                                    AluOp.subtract)
            nc.scalar.activation(rd[:], rd[:], mybir.ActivationFunctionType.Abs)
            nc.vector.tensor_tensor(rd[:], rd[:], padv01[:], AluOp.add)
            lg = big.tile([128, C], dt.float32)
            nc.scalar.activation(lg[:], rd[:], mybir.ActivationFunctionType.Ln)
            netE = big.tile([128, C], dt.float32)
            # netE = 0.5*(-0.298*(2.1 + 2.9823825*lg) + 5*|dist-2.04|)
            nc.vector.tensor_scalar(netE[:], lg[:],
                                    0.5 * -0.001 * TEMPERATURE * 2.9823825,
                                    0.5 * -0.001 * TEMPERATURE * 2.1,
                                    AluOp.mult, AluOp.add)
            nc.vector.tensor_scalar(tmp[:], dist[:], 2.04, None,
                                    AluOp.subtract)
            nc.scalar.activation(tmp[:], tmp[:], mybir.ActivationFunctionType.Abs)
            nc.vector.tensor_scalar(tmp[:], tmp[:], 2.5, None, AluOp.mult)
            nc.vector.tensor_tensor(netE[:], netE[:], tmp[:], AluOp.add)
            # alt bits -> contrib [128, C, 4]
            ab = big.tile([128, C], dt.int32)
            a0i = big.tile([128, C], dt.int32)
            a1i = big.tile([128, C], dt.int32)
            nc.vector.tensor_copy(a0i[:], rec0[:, :, 5])
            nc.vector.tensor_copy(a1i[:], rec1[:, :, 5])
            nc.vector.tensor_tensor(ab[:], a0i[:], a1i[:], AluOp.bitwise_and)
            contrib = big.tile([128, C, 4], dt.float32)
            for k in range(A):
                bk = sm.tile([128, C], dt.int32, tag="bk")
                bf = sm.tile([128, C], dt.float32, tag="bf")
                nc.vector.tensor_scalar(bk[:], ab[:], 1 << k, None,
                                        AluOp.bitwise_and)
                nc.vector.tensor_copy(bf[:], bk[:])
                nc.vector.tensor_scalar(bf[:], bf[:], 0.0, None, AluOp.not_equal)
                nc.vector.tensor_tensor(contrib[:, :, k], netE[:], bf[:],
                                        AluOp.mult)

            # ---- flat offsets (+pad fix: pads have p0f > N_ATOMS)
            # pads write to dump rows NBINS+p (sliced off on host)
            prow = big.tile([128, C], dt.float32)
            nc.vector.tensor_scalar(prow[:], padv01[:], prow_d_col[:], None,
                                    AluOp.mult)
            vm = big.tile([128, C], dt.float32)
            nc.vector.tensor_scalar(vm[:], padv01[:], -1.0, 1.0, AluOp.mult,
                                    AluOp.add)
            f0 = big.tile([128, C], dt.float32)
            f1 = big.tile([128, C], dt.float32)
            nc.vector.tensor_tensor(f0[:], rec0[:, :, 4], vm[:], AluOp.mult)
            nc.vector.tensor_tensor(f0[:], f0[:], prow[:], AluOp.add)
            nc.vector.tensor_tensor(f1[:], rec1[:, :, 4], vm[:], AluOp.mult)
            nc.vector.tensor_tensor(f1[:], f1[:], prow[:], AluOp.add)
            f0i = big.tile([128, C], dt.int32)
            f1i = big.tile([128, C], dt.int32)
            nc.vector.tensor_copy(f0i[:], f0[:])
            nc.vector.tensor_copy(f1i[:], f1[:])

            # ---- dedup totals via per-column group matmuls, then write scatters
            ma0 = big.tile([128, C * 128], dt.float32)
            ma1 = big.tile([128, C * 128], dt.float32)
            mr0 = big.tile([128, C * 128], dt.float32)
            mr1 = big.tile([128, C * 128], dt.float32)
            nc.sync.dma_start(ma0[:], ma0_d.ap())
            nc.sync.dma_start(ma1[:], ma1_d.ap())
            nc.sync.dma_start(mr0[:], mr0_d.ap())
            nc.sync.dma_start(mr1[:], mr1_d.ap())
            dbg = big.tile([128, C * 24], dt.float32)
            nc.vector.tensor_copy(dbg[:].rearrange("p (c e) -> p c e", e=24)[:, :, 0:8], rec0)
            nc.vector.tensor_copy(dbg[:].rearrange("p (c e) -> p c e", e=24)[:, :, 8:16], rec1)
            nc.vector.tensor_copy(dbg[:].rearrange("p (c e) -> p c e", e=24)[:, :, 16:17], netE[:].rearrange("p c -> p c ()"))
            nc.vector.tensor_copy(dbg[:].rearrange("p (c e) -> p c e", e=24)[:, :, 17:21], contrib[:])
            nc.vector.tensor_copy(dbg[:].rearrange("p (c e) -> p c e", e=24)[:, :, 21:22], rd[:].rearrange("p c -> p c ()"))
            nc.vector.tensor_copy(dbg[:].rearrange("p (c e) -> p c e", e=24)[:, :, 22:23], dist[:].rearrange("p c -> p c ()"))
            nc.sync.dma_start(dbg_d.ap(), dbg[:])
            tots = {}
            for name, m in (("a0", ma0), ("a1", ma1), ("r0", mr0), ("r1", mr1)):
                tot = big.tile([128, C * 4], dt.float32, tag="tot" + name)
                for c in range(C):
                    pt = ps.tile([128, 4], dt.float32, tag="pt")
                    nc.tensor.matmul(out=pt[:], lhsT=m[:, c * 128:(c + 1) * 128],
                                     rhs=contrib[:, c, :], start=True, stop=True)
                    nc.vector.tensor_copy(tot[:, c * 4:(c + 1) * 4], pt[:])
                tots[name] = tot
            for c in range(C):
                nc.gpsimd.indirect_dma_start(
                    ae_d.ap(), IndirectOffsetOnAxis(ap=p0i[:, c:c + 1], axis=0),
                    tots["a0"][:, c * 4:(c + 1) * 4], None)
                nc.gpsimd.indirect_dma_start(
                    ae_d.ap(), IndirectOffsetOnAxis(ap=p1i[:, c:c + 1], axis=0),
                    tots["a1"][:, c * 4:(c + 1) * 4], None)
                nc.gpsimd.indirect_dma_start(
                    re_d.ap(), IndirectOffsetOnAxis(ap=f0i[:, c:c + 1], axis=0),
                    tots["r0"][:, c * 4:(c + 1) * 4], None)
                nc.gpsimd.indirect_dma_start(
                    re_d.ap(), IndirectOffsetOnAxis(ap=f1i[:, c:c + 1], axis=0),
                    tots["r1"][:, c * 4:(c + 1) * 4], None)
    nc.compile()
    return nc


def _get_tail_program():
    if "tail" not in _prog_cache:
        _prog_cache["tail"] = _build_tail_program()
    return _prog_cache["tail"]


class _UF:
    def __init__(self, n):
        self.p = list(range(n))

    def find(self, x):
        while self.p[x] != x:
            self.p[x] = self.p[self.p[x]]
            x = self.p[x]
        return x

    def union(self, a, b):
        ra, rb = self.find(a), self.find(b)
        if ra != rb:
            self.p[ra] = rb


def _pack_tail_core(h0, h1, fl0, fl1):
    """Pack this core's hit pairs into [128, C_PAIR] columns such that no two
    columns share an atom id or flat id. Returns (rows, cols, ok)."""
    n = len(h0)
    C = C_PAIR
    if n == 0:
        return np.zeros(0, np.int64), np.zeros(0, np.int64), True
    uf = _UF(n)
    for keys in (np.concatenate([h0, h1]), np.concatenate([fl0, fl1])):
        pid = np.tile(np.arange(n), 2)
        o = np.argsort(keys, kind="stable")
        ks, po = keys[o], pid[o]
        same = np.flatnonzero(ks[1:] == ks[:-1])
        for i in same:
            uf.union(int(po[i]), int(po[i + 1]))
    root = np.array([uf.find(i) for i in range(n)])
    comps = {}
    for i, r in enumerate(root):
        comps.setdefault(r, []).append(i)
    # first-fit into C columns of capacity 128
    cap = [128] * C
    rows = np.empty(n, np.int64)
    cols = np.empty(n, np.int64)
    for members in sorted(comps.values(), key=len, reverse=True):
        sz = len(members)
        if sz > 128:
            return rows, cols, False
        placed = False
        for c in range(C):
            if cap[c] >= sz:
                r0 = 128 - cap[c]
                for j, m in enumerate(members):
                    rows[m] = r0 + j
                    cols[m] = c
                cap[c] -= sz
                placed = True
                break
        if not placed:
            return rows, cols, False
    return rows, cols, True


def _host_tail(coords, resnum, alt, p0, p1, hit):
    atomEnergy = np.zeros((N_ATOMS, A), np.float32)
    if len(hit):
        h0, h1 = p0[hit], p1[hit]
        diff = coords[h0] - coords[h1] + 1e-6
        dist = np.sqrt(np.sum(diff * diff, axis=-1))
        rd = np.abs(resnum[h0] - resnum[h1]).astype(np.float32)
        energy = -0.001 * TEMPERATURE * (2.1 + 2.9823825 * np.log(rd)) \
            + 5.0 * np.abs(dist - 2.04)
        netE = 0.5 * energy
        pair_alt = alt[h0] & alt[h1]
        contrib = np.where(pair_alt, netE[:, None], 0.0).astype(np.float32)
        np.add.at(atomEnergy, h0, contrib)
        np.add.at(atomEnergy, h1, contrib)
    return atomEnergy


def kernel(coords, atom_description, atom_number, atomPairs, alternativeMask,
           partners, facc):
    coords = np.asarray(coords, np.float32)
    ad = np.asarray(atom_description)
    pairs = np.asarray(atomPairs).astype(np.int64)
    alt = np.asarray(alternativeMask).astype(bool)
    at_name = ad[:, 3].astype(np.int64)
    resnum = ad[:, 2].astype(np.int64)
    p0, p1 = pairs[:, 0], pairs[:, 1]
    npair = len(p0)

    at_u8 = at_name.astype(np.uint8)

    # choose T adaptively (worst-case chunk count over both sides)
    def n_chunks_needed(p):
        win_counts = np.bincount(p // WIN)
        wc = win_counts[win_counts > 0]
        return int(np.sum((wc + F - 1) // F))

    need = max(n_chunks_needed(p0), n_chunks_needed(p1))
    T = max(1, -(-need // (8 * 128)))
    nc1 = _get_program(T, 1)
    nc2 = _get_program(T, 2)

    # weights for bitmap packing
    k = np.arange(32)
    w_lo = np.where(k < 16, 2.0 ** k, 0.0)
    w_hi = np.where(k >= 16, 2.0 ** (k - 16), 0.0)
    wts = np.tile((w_lo + w_hi).astype(np.float32), T)[None, :].repeat(128, 0)
    wts[:, 31::32] = 0.0

    # ---- phase 1: p1 side
    slot1, pf1, basem1, boc1, _ = _build_layout(p1, T)
    anw1 = _build_anw(at_u8, boc1)
    res1 = _run_phase(nc1, pf1, basem1, anw1, wts)
    m1 = np.stack([res1.results[c]["mby"] for c in range(N_CORES)])  # [8,128,TF]
    m1_of_pair = m1.reshape(-1)[slot1]

    # ---- relay into p0 layout ({0,1} int8; phase 2 fuses ne + logical_and)
    slot0, pf0, basem0, boc0, _ = _build_layout(p0, T)
    anw0 = _build_anw(at_u8, boc0)
    mfac0 = np.zeros(8 * 128 * T * F, np.int8)
    mfac0[slot0] = m1_of_pair
    res0 = _run_phase(nc2, pf0, basem0, anw0, wts,
                      mfac0.reshape(8, 128, T * F))
    sby = np.stack([res0.results[c]["mby"] for c in range(N_CORES)])
    sulfur = sby.reshape(-1)[slot0] != 0

    # ---- sparse tail on device (phase 3); host fallback for over-capacity
    flat = ((ad[:, 0].astype(np.int64) * N_CHAIN + ad[:, 1]) * N_RES + resnum)
    hit = np.flatnonzero(sulfur)
    # Device tail by default; host fallback only for over-capacity inputs
    # (>128*C_PAIR sulfur pairs per core) or if explicitly disabled.
    import os
    use_device_tail = (os.environ.get("DISULF_DEVICE_TAIL", "1") == "1"
                       and len(hit) <= N_CORES * 128 * C_PAIR)

    if use_device_tail:
        from concourse import bass_utils
        nct = _get_tail_program()
        C = C_PAIR
        # shared per-atom inputs, padded to 128*REC_P
        NP_ = 128 * REC_P
        co_pad = np.zeros((NP_, 3), np.float32)
        co_pad[:N_ATOMS] = coords
        adf_pad = np.zeros((NP_, 3), np.float32)
        adf_pad[:N_ATOMS, 0] = ad[:, 0]
        adf_pad[:N_ATOMS, 1] = ad[:, 1]
        adf_pad[:N_ATOMS, 2] = resnum
        alb_pad = np.zeros(NP_, np.float32)
        alb_pad[:N_ATOMS] = (alt * (1 << np.arange(4))).sum(1)
        co_in = co_pad.reshape(128, REC_P * 3)
        adf_in = adf_pad.reshape(128, REC_P * 3)
        alb_in = alb_pad.reshape(128, REC_P)
        # split hits evenly across cores
        in_maps = []
        per = [hit[c::N_CORES] for c in range(N_CORES)]
        ok_all = True
        for c in range(N_CORES):
            hc = per[c]
            h0, h1 = p0[hc], p1[hc]
            f0, f1 = flat[h0], flat[h1]
            rows, cols, ok = _pack_tail_core(h0, h1, f0, f1)
            ok_all &= ok
            if not ok:
                break
            rowpad = (N_ATOMS + np.arange(128, dtype=np.int32))[:, None]
            p0i = np.broadcast_to(rowpad, (128, C)).astype(np.int32).copy()
            p1i = p0i.copy()
            # pad ids N_ATOMS+p: never equal a real atom; within a column all distinct
            a0 = p0i.astype(np.int64).copy()
            a1 = a0.copy()
            g0 = np.broadcast_to(NBINS + np.arange(128)[:, None],
                                 (128, C)).astype(np.int64).copy()
            g1 = g0.copy()
            a0[rows, cols] = h0
            a1[rows, cols] = h1
            g0[rows, cols] = f0
            g1[rows, cols] = f1
            p0i[rows, cols] = h0
            p1i[rows, cols] = h1
            ma0 = np.zeros((128, C, 128), np.float32)
            ma1 = np.zeros((128, C, 128), np.float32)
            mr0 = np.zeros((128, C, 128), np.float32)
            mr1 = np.zeros((128, C, 128), np.float32)
            for cc in range(C):
                A0, A1 = a0[:, cc], a1[:, cc]
                G0, G1 = g0[:, cc], g1[:, cc]
                ma0[:, cc, :] = ((A0[:, None] == A0[None, :]) +
                                 (A1[:, None] == A0[None, :])).astype(np.float32)
                ma1[:, cc, :] = ((A0[:, None] == A1[None, :]) +
                                 (A1[:, None] == A1[None, :])).astype(np.float32)
                mr0[:, cc, :] = ((G0[:, None] == G0[None, :]) +
                                 (G1[:, None] == G0[None, :])).astype(np.float32)
                mr1[:, cc, :] = ((G0[:, None] == G1[None, :]) +
                                 (G1[:, None] == G1[None, :])).astype(np.float32)
            in_maps.append(dict(
                co=co_in, adf=adf_in, alb=alb_in,
                p0i=p0i, p1i=p1i, p0f=p0i.astype(np.float32),
                ma0=ma0.reshape(128, -1), ma1=ma1.reshape(128, -1),
                mr0=mr0.reshape(128, -1), mr1=mr1.reshape(128, -1),
                prow=(NBINS + np.arange(128, dtype=np.float32))[:, None]))
        if ok_all:
            rest = bass_utils.run_bass_kernel_spmd(
                nct, in_maps, core_ids=list(range(N_CORES)))
            atomEnergy = np.zeros((N_ATOMS, A), np.float32)
            residueEnergy = np.zeros((NBINS, A), np.float32)
            for c in range(N_CORES):
                atomEnergy += rest.results[c]["ae"][:N_ATOMS]
                residueEnergy += rest.results[c]["re"][:NBINS]
        else:
            use_device_tail = False

    if not use_device_tail:
        atomEnergy = _host_tail(coords, resnum, alt, p0, p1, hit)
        residueEnergy = np.zeros((NBINS, A), np.float32)
        nz = np.flatnonzero(np.any(atomEnergy != 0, axis=1))
        np.add.at(residueEnergy, flat[nz], atomEnergy[nz])

    residueEnergy = residueEnergy.reshape(N_BATCH, N_CHAIN, N_RES, A)
    return residueEnergy, atomEnergy, sulfur


# revision 20
# speedup vs baseline: 1.8545x; 1.2000x over previous
"""Trainium2 Bass kernel for nn_Disulfide_net (edge-parallel GNN scatter).

Strategy (8 NeuronCores, SPMD):
  - Host shards atomPairs by sorting each endpoint column and grouping pairs
    into "chunks" of <=F pairs whose endpoint atoms span a <=31-atom window
    (graph-partition sharding + per-chunk halo of at_name bytes).
  - Device phase 1 (p1-sorted layout): per chunk, pack isSG (at_name==16) of
    the 31-atom window into an int32 bitmap held as a per-partition scalar;
    each pair tests its bit by ANDing the host-supplied 2^(p%31) selector
    against the bitmap -> m1 mask for all 8M pairs in 2 fused DVE passes
    (~1/64 cycle per pair).
  - Host relays m1 into the p0-sorted layout (pure index permutation).
  - Device phase 2 (p0-sorted layout): m0 bit-test AND m1 -> sulfur mask.
  - Device phase 3 (sparse tail, ~1/1024 of pairs): indirect-DMA gathers of
    per-atom records, dense energy evaluation, duplicate-group totals via
    per-column PE matmuls (host supplies 0/1 group matrices from pure index
    data), then indirect-DMA row writes into per-core atomEnergy /
    residueEnergy partials; host sums the 8 partials on unshard.
"""
import sys
import numpy as np

sys.path.insert(0, "/opt/trn_rl_repo")

N_CORES = 8
N_ATOMS = 500_000
N_PAIRS = 8_000_000
A = 4
N_BATCH, N_CHAIN, N_RES = 2, 4, 50000
SG_HASH = 16
TEMPERATURE = 298.0
WIN = 31          # atoms per chunk window (bit 31 of int32 never used)
F = 576           # pair slots per chunk (~16% pad at lambda~496)
_prog_cache = {}


def _build_layout(pvals, T):
    """Assign each pair a slot in the [8, 128, T, F] chunked layout, sorted by
    its endpoint atom id. Returns (slot_of_pair, per-core arrays)."""
    order = np.argsort(pvals, kind="stable")
    ps = pvals[order]
    win = ps // WIN
    uw, first = np.unique(win, return_index=True)
    cnts = np.diff(np.r_[first, len(ps)])
    nchunk_per_win = (cnts + F - 1) // F
    chunk_base = np.r_[0, np.cumsum(nchunk_per_win)][:-1]
    n_chunks = int(chunk_base[-1] + nchunk_per_win[-1]) if len(uw) else 0
    assert n_chunks <= 8 * 128 * T, f"{n_chunks} chunks > capacity {8*128*T}"
    rank = np.arange(len(ps)) - np.repeat(first, cnts)
    chunk_of = np.repeat(chunk_base, cnts) + rank // F
    slot_in = rank % F
    # chunk c -> core = c // (128*T), p = (c % (128*T)) % 128, t = (...) // 128
    c_loc = chunk_of % (128 * T)
    # [core, p, t, f] flat = ((core*128 + p)*T + t)*F + f
    slot = ((chunk_of // (128 * T)) * 128 + (c_loc % 128)) * (T * F) \
        + (c_loc // 128) * F + slot_in
    slot_of_pair = np.empty(len(ps), np.int64)
    slot_of_pair[order] = slot
    # per-chunk metadata, scattered into the [core, p, t] layout order
    cid = np.arange(n_chunks)
    cl = cid % (128 * T)
    lay = ((cid // (128 * T)) * 128 + (cl % 128)) * T + (cl // 128)
    base_of_chunk = np.zeros(8 * 128 * T, np.int64)
    base_of_chunk[lay] = np.repeat(uw, nchunk_per_win) * WIN
    # host ships the bit-select value 2^(p mod WIN) directly (pure layout
    # data); pad slots keep 0 -> mask 0 with no further handling
    pw = np.zeros((8, 128, T, F), np.int32)
    pw.reshape(-1)[slot] = (np.int32(1) << (ps % WIN).astype(np.int32))
    return slot_of_pair, pw, base_of_chunk.reshape(8, 128, T), n_chunks


def _build_anw(at_name_u8, base_of_chunk):
    """Per-chunk 32-byte at_name halo windows. [8,128,T,32] int8."""
    idx = base_of_chunk[..., None] + np.arange(32)
    valid = idx < N_ATOMS
    idxc = np.clip(idx, 0, N_ATOMS - 1)
    anw = at_name_u8[idxc]
    anw[~valid] = 0
    anw[..., 31] = 0  # bit31 unused
    return anw.astype(np.int8)


def _build_program(T, phase):
    import concourse.bacc as bacc
    import concourse.mybir as mybir
    from concourse.tile import TileContext
    dt = mybir.dt
    AluOp = mybir.AluOpType

    nc = bacc.Bacc("TRN2", target_bir_lowering=False, debug=False,
                   num_devices=N_CORES)
    TF = T * F
    pw_d = nc.dram_tensor("pw", [128, TF], dt.int32, kind="ExternalInput")
    anw_d = nc.dram_tensor("anw", [128, T * 32], dt.int8, kind="ExternalInput")
    wts_d = nc.dram_tensor("wts", [128, T * 32], dt.float32, kind="ExternalInput")
    if phase == 2:
        mfac_d = nc.dram_tensor("mfac", [128, TF], dt.int8, kind="ExternalInput")
    mby_d = nc.dram_tensor("mby", [128, TF], dt.int8, kind="ExternalOutput")

    with TileContext(nc) as tc:
        with tc.tile_pool(name="big", bufs=1) as big, \
             tc.tile_pool(name="small", bufs=3) as small:
            pwt = big.tile([128, TF], dt.int32)
            mby = big.tile([128, TF], dt.int8)
            anw = big.tile([128, T * 32], dt.int8)
            wts = big.tile([128, T * 32], dt.float32)
            nc.sync.dma_start(pwt[:], pw_d.ap())
            nc.sync.dma_start(anw[:], anw_d.ap())
            nc.sync.dma_start(wts[:], wts_d.ap())
            if phase == 2:
                mfac = big.tile([128, TF], dt.int8)
                nc.sync.dma_start(mfac[:], mfac_d.ap())

            # ---- bitmap build: B[p, t] int32 of isSG over the chunk window
            anf = big.tile([128, T * 32], dt.float32)
            nc.vector.tensor_copy(anf[:], anw[:])
            eq = big.tile([128, T * 32], dt.float32)
            nc.vector.tensor_scalar(eq[:], anf[:], float(SG_HASH), None,
                                    AluOp.is_equal)
            nc.vector.tensor_tensor(eq[:], eq[:], wts[:], AluOp.mult)
            eq3 = eq[:].rearrange("p (t k) -> p t k", k=32)
            lo = big.tile([128, T], dt.float32)
            hi = big.tile([128, T], dt.float32)
            nc.vector.tensor_reduce(lo[:], eq3[:, :, 0:16],
                                    axis=mybir.AxisListType.X, op=AluOp.add)
            nc.vector.tensor_reduce(hi[:], eq3[:, :, 16:32],
                                    axis=mybir.AxisListType.X, op=AluOp.add)
            loi = big.tile([128, T], dt.int32)
            hii = big.tile([128, T], dt.int32)
            nc.vector.tensor_copy(loi[:], lo[:])
            nc.vector.tensor_copy(hii[:], hi[:])
            nc.vector.tensor_scalar(hii[:], hii[:], 16, None,
                                    AluOp.logical_shift_left)
            B = big.tile([128, T], dt.int32)
            nc.vector.tensor_tensor(B[:], loi[:], hii[:], AluOp.bitwise_or)

            # ---- per-tile mask: 2 fused passes (2^w supplied by host)
            for t in range(T):
                sl = slice(t * F, (t + 1) * F)
                mr = small.tile([128, F], dt.int32, tag="mr")
                nc.vector.tensor_scalar(mr[:], pwt[:, sl], B[:, t:t + 1], None,
                                        AluOp.bitwise_and)
                if phase == 2:
                    # (bit != 0) AND m1 {0,1} int8 relay
                    nc.vector.scalar_tensor_tensor(
                        mby[:, sl], mr[:], 0, mfac[:, sl],
                        AluOp.not_equal, AluOp.logical_and)
                else:
                    nc.vector.tensor_scalar(mby[:, sl], mr[:], 0, None,
                                            AluOp.not_equal)
            nc.sync.dma_start(mby_d.ap(), mby[:])
    nc.compile()
    return nc

def _get_program(T, phase):
    key = (T, phase)
    if key not in _prog_cache:
        _prog_cache[key] = _build_program(T, phase)
    return _prog_cache[key]


def _run_phase(nc, pw, anw, wts, mfac=None):
    from concourse import bass_utils
    in_maps = []
    for c in range(N_CORES):
        m = dict(pw=np.ascontiguousarray(pw[c].reshape(128, -1)),
                 anw=np.ascontiguousarray(anw[c].reshape(128, -1)),
                 wts=wts)
        if mfac is not None:
            m["mfac"] = np.ascontiguousarray(mfac[c].reshape(128, -1))
        in_maps.append(m)
    res = bass_utils.run_bass_kernel_spmd(nc, in_maps,
                                          core_ids=list(range(N_CORES)))
    return res


C_PAIR = 8            # pair columns per core in the tail (capacity 1024 pairs)
REC_P = 3936          # 128*3936 = 503808 >= N_ATOMS
NBINS = N_BATCH * N_CHAIN * N_RES


def _build_tail_program():
    import concourse.bacc as bacc
    import concourse.mybir as mybir
    from concourse.tile import TileContext
    dt = mybir.dt
    AluOp = mybir.AluOpType
    AF = mybir.ActivationFunctionType
    from concourse.bass import IndirectOffsetOnAxis

    C = C_PAIR
    nc = bacc.Bacc("TRN2", target_bir_lowering=False, debug=False,
                   num_devices=N_CORES)
    co_d = nc.dram_tensor("co", [128, REC_P * 3], dt.float32, kind="ExternalInput")
    adf_d = nc.dram_tensor("adf", [128, REC_P * 3], dt.float32, kind="ExternalInput")
    alb_d = nc.dram_tensor("alb", [128, REC_P], dt.float32, kind="ExternalInput")
    p0i_d = nc.dram_tensor("p0i", [128, C], dt.int32, kind="ExternalInput")
    p1i_d = nc.dram_tensor("p1i", [128, C], dt.int32, kind="ExternalInput")
    p0f_d = nc.dram_tensor("p0f", [128, C], dt.float32, kind="ExternalInput")
    ma0_d = nc.dram_tensor("ma0", [128, C * 128], dt.float32, kind="ExternalInput")
    ma1_d = nc.dram_tensor("ma1", [128, C * 128], dt.float32, kind="ExternalInput")
    mr0_d = nc.dram_tensor("mr0", [128, C * 128], dt.float32, kind="ExternalInput")
    mr1_d = nc.dram_tensor("mr1", [128, C * 128], dt.float32, kind="ExternalInput")
    prow_d = nc.dram_tensor("prow", [128, 1], dt.float32, kind="ExternalInput")
    ae_d = nc.dram_tensor("ae", [N_ATOMS + 128, A], dt.float32, kind="ExternalOutput")
    re_d = nc.dram_tensor("re", [NBINS + 128, A], dt.float32, kind="ExternalOutput")
    dbg_d = nc.dram_tensor("dbg", [128, C * 24], dt.float32, kind="ExternalOutput")
    dbg2_d = nc.dram_tensor("dbg2", [1, 128], dt.float32, kind="ExternalOutput")
    rec_t = nc.dram_tensor("rectab", [128 * REC_P, 8], dt.float32, kind="Internal")

    with TileContext(nc) as tc:
        with tc.tile_pool(name="big", bufs=1) as big, \
             tc.tile_pool(name="sm", bufs=2) as sm, \
             tc.tile_pool(name="ps", bufs=2, space="PSUM") as ps:
            rec_dram = rec_t
            # ---- build fat record table [atom, 8] = x,y,z,resnum,flat,altbits,0,0
            NSL = 4
            SL = REC_P // NSL
            for s in range(NSL):
                co = sm.tile([128, SL, 3], dt.float32, tag="co")
                adf = sm.tile([128, SL, 3], dt.float32, tag="adf")
                alb = sm.tile([128, SL], dt.float32, tag="alb")
                rec = sm.tile([128, SL, 8], dt.float32, tag="rec")
                nc.sync.dma_start(co[:], co_d.ap().rearrange(
                    "p (n c) -> p n c", c=3)[:, s * SL:(s + 1) * SL, :])
                nc.sync.dma_start(adf[:], adf_d.ap().rearrange(
                    "p (n c) -> p n c", c=3)[:, s * SL:(s + 1) * SL, :])
                nc.sync.dma_start(alb[:], alb_d.ap()[:, s * SL:(s + 1) * SL])
                for k in range(3):
                    nc.vector.tensor_copy(rec[:, :, k], co[:, :, k])
                nc.vector.tensor_copy(rec[:, :, 3], adf[:, :, 2])
                # flat = b*200000 + c*50000 + r
                nc.vector.tensor_scalar(rec[:, :, 4], adf[:, :, 0], 200000.0,
                                        None, AluOp.mult)
                fl2 = sm.tile([128, SL], dt.float32, tag="fl2")
                nc.vector.tensor_scalar(fl2[:], adf[:, :, 1], 50000.0, None,
                                        AluOp.mult)
                nc.vector.tensor_tensor(rec[:, :, 4], rec[:, :, 4], fl2[:],
                                        AluOp.add)
                nc.vector.tensor_tensor(rec[:, :, 4], rec[:, :, 4],
                                        adf[:, :, 2], AluOp.add)
                nc.vector.tensor_copy(rec[:, :, 5], alb[:])
                nc.gpsimd.memset(rec[:, :, 6:8], 0.0)
                nc.sync.dma_start(
                    rec_dram.ap().rearrange("(p n) e -> p n e", p=128)
                    [:, s * SL:(s + 1) * SL, :], rec[:])

            dbg2 = big.tile([1, 128], dt.float32)
            nc.sync.dma_start(dbg2[:], rec_t.ap()[0:16, :].rearrange("r e -> () (r e)"))
            nc.sync.dma_start(dbg2_d.ap(), dbg2[:])
            p0i = big.tile([128, C], dt.int32)
            p1i = big.tile([128, C], dt.int32)
            p0f = big.tile([128, C], dt.float32)
            prow_d_col = big.tile([128, 1], dt.float32)
            nc.sync.dma_start(p0i[:], p0i_d.ap())
            nc.sync.dma_start(p1i[:], p1i_d.ap())
            nc.sync.dma_start(p0f[:], p0f_d.ap())
            nc.sync.dma_start(prow_d_col[:], prow_d.ap())
            rec0f = big.tile([128, C * 8], dt.float32)
            rec1f = big.tile([128, C * 8], dt.float32)
            nc.gpsimd.memset(rec0f[:], 0.0)
            nc.gpsimd.memset(rec1f[:], 0.0)
            for c in range(C):
                nc.gpsimd.indirect_dma_start(
                    rec0f[:, c * 8:(c + 1) * 8], None, rec_dram.ap(),
                    IndirectOffsetOnAxis(ap=p0i[:, c:c + 1], axis=0))
                nc.gpsimd.indirect_dma_start(
                    rec1f[:, c * 8:(c + 1) * 8], None, rec_dram.ap(),
                    IndirectOffsetOnAxis(ap=p1i[:, c:c + 1], axis=0))
            rec0 = rec0f[:].rearrange("p (c e) -> p c e", e=8)
            rec1 = rec1f[:].rearrange("p (c e) -> p c e", e=8)

            # ---- energy per pair slot [128, C]
            d2 = big.tile([128, C], dt.float32)
            tmp = big.tile([128, C], dt.float32)
            nc.gpsimd.memset(d2[:], 0.0)
            for k in range(3):
                dx = sm.tile([128, C], dt.float32, tag="dx")
                nc.vector.tensor_tensor(dx[:], rec0[:, :, k], rec1[:, :, k],
                                        AluOp.subtract)
                nc.vector.tensor_scalar(dx[:], dx[:], 1e-6, None, AluOp.add)
                nc.vector.tensor_tensor(dx[:], dx[:], dx[:], AluOp.mult)
                nc.vector.tensor_tensor(d2[:], d2[:], dx[:], AluOp.add)
            dist = big.tile([128, C], dt.float32)
            nc.scalar.activation(dist[:], d2[:], mybir.ActivationFunctionType.Sqrt)
            padv01 = big.tile([128, C], dt.float32)
            nc.vector.tensor_scalar(padv01[:], p0f[:], float(N_ATOMS) - 0.5,
                                    None, AluOp.is_gt)
            rd = big.tile([128, C], dt.float32)
            nc.vector.tensor_tensor(rd[:], rec0[:, :, 3], rec1[:, :, 3],
                                    AluOp.subtract)
            nc.scalar.activation(rd[:], rd[:], mybir.ActivationFunctionType.Abs)
            nc.vector.tensor_tensor(rd[:], rd[:], padv01[:], AluOp.add)
            lg = big.tile([128, C], dt.float32)
            nc.scalar.activation(lg[:], rd[:], mybir.ActivationFunctionType.Ln)
            netE = big.tile([128, C], dt.float32)
            # netE = 0.5*(-0.298*(2.1 + 2.9823825*lg) + 5*|dist-2.04|)
            nc.vector.tensor_scalar(netE[:], lg[:],
                                    0.5 * -0.001 * TEMPERATURE * 2.9823825,
                                    0.5 * -0.001 * TEMPERATURE * 2.1,
                                    AluOp.mult, AluOp.add)
            nc.vector.tensor_scalar(tmp[:], dist[:], 2.04, None,
                                    AluOp.subtract)
            nc.scalar.activation(tmp[:], tmp[:], mybir.ActivationFunctionType.Abs)
            nc.vector.tensor_scalar(tmp[:], tmp[:], 2.5, None, AluOp.mult)
            nc.vector.tensor_tensor(netE[:], netE[:], tmp[:], AluOp.add)
            # alt bits -> contrib [128, C, 4]
            ab = big.tile([128, C], dt.int32)
            a0i = big.tile([128, C], dt.int32)
            a1i = big.tile([128, C], dt.int32)
            nc.vector.tensor_copy(a0i[:], rec0[:, :, 5])
            nc.vector.tensor_copy(a1i[:], rec1[:, :, 5])
            nc.vector.tensor_tensor(ab[:], a0i[:], a1i[:], AluOp.bitwise_and)
            contrib = big.tile([128, C, 4], dt.float32)
            for k in range(A):
                bk = sm.tile([128, C], dt.int32, tag="bk")
                bf = sm.tile([128, C], dt.float32, tag="bf")
                nc.vector.tensor_scalar(bk[:], ab[:], 1 << k, None,
                                        AluOp.bitwise_and)
                nc.vector.tensor_copy(bf[:], bk[:])
                nc.vector.tensor_scalar(bf[:], bf[:], 0.0, None, AluOp.not_equal)
                nc.vector.tensor_tensor(contrib[:, :, k], netE[:], bf[:],
                                        AluOp.mult)

            # ---- flat offsets (+pad fix: pads have p0f > N_ATOMS)
            # pads write to dump rows NBINS+p (sliced off on host)
            prow = big.tile([128, C], dt.float32)
            nc.vector.tensor_scalar(prow[:], padv01[:], prow_d_col[:], None,
                                    AluOp.mult)
            vm = big.tile([128, C], dt.float32)
            nc.vector.tensor_scalar(vm[:], padv01[:], -1.0, 1.0, AluOp.mult,
                                    AluOp.add)
            f0 = big.tile([128, C], dt.float32)
            f1 = big.tile([128, C], dt.float32)
            nc.vector.tensor_tensor(f0[:], rec0[:, :, 4], vm[:], AluOp.mult)
            nc.vector.tensor_tensor(f0[:], f0[:], prow[:], AluOp.add)
            nc.vector.tensor_tensor(f1[:], rec1[:, :, 4], vm[:], AluOp.mult)
            nc.vector.tensor_tensor(f1[:], f1[:], prow[:], AluOp.add)
            f0i = big.tile([128, C], dt.int32)
            f1i = big.tile([128, C], dt.int32)
            nc.vector.tensor_copy(f0i[:], f0[:])
            nc.vector.tensor_copy(f1i[:], f1[:])

            # ---- dedup totals via per-column group matmuls, then write scatters
            ma0 = big.tile([128, C * 128], dt.float32)
            ma1 = big.tile([128, C * 128], dt.float32)
            mr0 = big.tile([128, C * 128], dt.float32)
            mr1 = big.tile([128, C * 128], dt.float32)
            nc.sync.dma_start(ma0[:], ma0_d.ap())
            nc.sync.dma_start(ma1[:], ma1_d.ap())
            nc.sync.dma_start(mr0[:], mr0_d.ap())
            nc.sync.dma_start(mr1[:], mr1_d.ap())
            dbg = big.tile([128, C * 24], dt.float32)
            nc.vector.tensor_copy(dbg[:].rearrange("p (c e) -> p c e", e=24)[:, :, 0:8], rec0)
            nc.vector.tensor_copy(dbg[:].rearrange("p (c e) -> p c e", e=24)[:, :, 8:16], rec1)
            nc.vector.tensor_copy(dbg[:].rearrange("p (c e) -> p c e", e=24)[:, :, 16:17], netE[:].rearrange("p c -> p c ()"))
            nc.vector.tensor_copy(dbg[:].rearrange("p (c e) -> p c e", e=24)[:, :, 17:21], contrib[:])
            nc.vector.tensor_copy(dbg[:].rearrange("p (c e) -> p c e", e=24)[:, :, 21:22], rd[:].rearrange("p c -> p c ()"))
            nc.vector.tensor_copy(dbg[:].rearrange("p (c e) -> p c e", e=24)[:, :, 22:23], dist[:].rearrange("p c -> p c ()"))
            nc.sync.dma_start(dbg_d.ap(), dbg[:])
            tots = {}
            for name, m in (("a0", ma0), ("a1", ma1), ("r0", mr0), ("r1", mr1)):
                tot = big.tile([128, C * 4], dt.float32, tag="tot" + name)
                for c in range(C):
                    pt = ps.tile([128, 4], dt.float32, tag="pt")
                    nc.tensor.matmul(out=pt[:], lhsT=m[:, c * 128:(c + 1) * 128],
                                     rhs=contrib[:, c, :], start=True, stop=True)
                    nc.vector.tensor_copy(tot[:, c * 4:(c + 1) * 4], pt[:])
                tots[name] = tot
            for c in range(C):
                nc.gpsimd.indirect_dma_start(
                    ae_d.ap(), IndirectOffsetOnAxis(ap=p0i[:, c:c + 1], axis=0),
                    tots["a0"][:, c * 4:(c + 1) * 4], None)
                nc.gpsimd.indirect_dma_start(
                    ae_d.ap(), IndirectOffsetOnAxis(ap=p1i[:, c:c + 1], axis=0),
                    tots["a1"][:, c * 4:(c + 1) * 4], None)
                nc.gpsimd.indirect_dma_start(
                    re_d.ap(), IndirectOffsetOnAxis(ap=f0i[:, c:c + 1], axis=0),
                    tots["r0"][:, c * 4:(c + 1) * 4], None)
                nc.gpsimd.indirect_dma_start(
                    re_d.ap(), IndirectOffsetOnAxis(ap=f1i[:, c:c + 1], axis=0),
                    tots["r1"][:, c * 4:(c + 1) * 4], None)
    nc.compile()
    return nc


def _get_tail_program():
    if "tail" not in _prog_cache:
        _prog_cache["tail"] = _build_tail_program()
    return _prog_cache["tail"]


class _UF:
    def __init__(self, n):
        self.p = list(range(n))

    def find(self, x):
        while self.p[x] != x:
            self.p[x] = self.p[self.p[x]]
            x = self.p[x]
        return x

    def union(self, a, b):
        ra, rb = self.find(a), self.find(b)
        if ra != rb:
            self.p[ra] = rb


def _pack_tail_core(h0, h1, fl0, fl1):
    """Pack this core's hit pairs into [128, C_PAIR] columns such that no two
    columns share an atom id or flat id. Returns (rows, cols, ok)."""
    n = len(h0)
    C = C_PAIR
    if n == 0:
        return np.zeros(0, np.int64), np.zeros(0, np.int64), True
    uf = _UF(n)
    for keys in (np.concatenate([h0, h1]), np.concatenate([fl0, fl1])):
        pid = np.tile(np.arange(n), 2)
        o = np.argsort(keys, kind="stable")
        ks, po = keys[o], pid[o]
        same = np.flatnonzero(ks[1:] == ks[:-1])
        for i in same:
            uf.union(int(po[i]), int(po[i + 1]))
    root = np.array([uf.find(i) for i in range(n)])
    comps = {}
    for i, r in enumerate(root):
        comps.setdefault(r, []).append(i)
    # first-fit into C columns of capacity 128
    cap = [128] * C
    rows = np.empty(n, np.int64)
    cols = np.empty(n, np.int64)
    for members in sorted(comps.values(), key=len, reverse=True):
        sz = len(members)
        if sz > 128:
            return rows, cols, False
        placed = False
        for c in range(C):
            if cap[c] >= sz:
                r0 = 128 - cap[c]
                for j, m in enumerate(members):
                    rows[m] = r0 + j
                    cols[m] = c
                cap[c] -= sz
                placed = True
                break
        if not placed:
            return rows, cols, False
    return rows, cols, True


def _host_tail(coords, resnum, alt, p0, p1, hit):
    atomEnergy = np.zeros((N_ATOMS, A), np.float32)
    if len(hit):
        h0, h1 = p0[hit], p1[hit]
        diff = coords[h0] - coords[h1] + 1e-6
        dist = np.sqrt(np.sum(diff * diff, axis=-1))
        rd = np.abs(resnum[h0] - resnum[h1]).astype(np.float32)
        energy = -0.001 * TEMPERATURE * (2.1 + 2.9823825 * np.log(rd)) \
            + 5.0 * np.abs(dist - 2.04)
        netE = 0.5 * energy
        pair_alt = alt[h0] & alt[h1]
        contrib = np.where(pair_alt, netE[:, None], 0.0).astype(np.float32)
        np.add.at(atomEnergy, h0, contrib)
        np.add.at(atomEnergy, h1, contrib)
    return atomEnergy


def kernel(coords, atom_description, atom_number, atomPairs, alternativeMask,
           partners, facc):
    coords = np.asarray(coords, np.float32)
    ad = np.asarray(atom_description)
    pairs = np.asarray(atomPairs).astype(np.int64)
    alt = np.asarray(alternativeMask).astype(bool)
    at_name = ad[:, 3].astype(np.int64)
    resnum = ad[:, 2].astype(np.int64)
    p0, p1 = pairs[:, 0], pairs[:, 1]
    npair = len(p0)

    at_u8 = at_name.astype(np.uint8)

    # choose T adaptively (worst-case chunk count over both sides)
    def n_chunks_needed(p):
        win_counts = np.bincount(p // WIN)
        wc = win_counts[win_counts > 0]
        return int(np.sum((wc + F - 1) // F))

    need = max(n_chunks_needed(p0), n_chunks_needed(p1))
    T = max(1, -(-need // (8 * 128)))
    nc1 = _get_program(T, 1)
    nc2 = _get_program(T, 2)

    # weights for bitmap packing
    k = np.arange(32)
    w_lo = np.where(k < 16, 2.0 ** k, 0.0)
    w_hi = np.where(k >= 16, 2.0 ** (k - 16), 0.0)
    wts = np.tile((w_lo + w_hi).astype(np.float32), T)[None, :].repeat(128, 0)
    wts[:, 31::32] = 0.0

    # ---- phase 1: p1 side
    slot1, pw1, boc1, _ = _build_layout(p1, T)
    anw1 = _build_anw(at_u8, boc1)
    res1 = _run_phase(nc1, pw1, anw1, wts)
    m1 = np.stack([res1.results[c]["mby"] for c in range(N_CORES)])  # [8,128,TF]
    m1_of_pair = m1.reshape(-1)[slot1]

    # ---- relay into p0 layout ({0,1} int8; phase 2 fuses ne + logical_and)
    slot0, pw0, boc0, _ = _build_layout(p0, T)
    anw0 = _build_anw(at_u8, boc0)
    mfac0 = np.zeros(8 * 128 * T * F, np.int8)
    mfac0[slot0] = m1_of_pair
    res0 = _run_phase(nc2, pw0, anw0, wts,
                      mfac0.reshape(8, 128, T * F))
    sby = np.stack([res0.results[c]["mby"] for c in range(N_CORES)])
    sulfur = sby.reshape(-1)[slot0] != 0

    # ---- sparse tail on device (phase 3); host fallback for over-capacity
    flat = ((ad[:, 0].astype(np.int64) * N_CHAIN + ad[:, 1]) * N_RES + resnum)
    hit = np.flatnonzero(sulfur)
    # Device tail by default; host fallback only for over-capacity inputs
    # (>128*C_PAIR sulfur pairs per core) or if explicitly disabled.
    import os
    use_device_tail = (os.environ.get("DISULF_DEVICE_TAIL", "1") == "1"
                       and len(hit) <= N_CORES * 128 * C_PAIR)

    if use_device_tail:
        from concourse import bass_utils
        nct = _get_tail_program()
        C = C_PAIR
        # shared per-atom inputs, padded to 128*REC_P
        NP_ = 128 * REC_P
        co_pad = np.zeros((NP_, 3), np.float32)
        co_pad[:N_ATOMS] = coords
        adf_pad = np.zeros((NP_, 3), np.float32)
        adf_pad[:N_ATOMS, 0] = ad[:, 0]
        adf_pad[:N_ATOMS, 1] = ad[:, 1]
        adf_pad[:N_ATOMS, 2] = resnum
        alb_pad = np.zeros(NP_, np.float32)
        alb_pad[:N_ATOMS] = (alt * (1 << np.arange(4))).sum(1)
        co_in = co_pad.reshape(128, REC_P * 3)
        adf_in = adf_pad.reshape(128, REC_P * 3)
        alb_in = alb_pad.reshape(128, REC_P)
        # split hits evenly across cores
        in_maps = []
        per = [hit[c::N_CORES] for c in range(N_CORES)]
        ok_all = True
        for c in range(N_CORES):
            hc = per[c]
            h0, h1 = p0[hc], p1[hc]
            f0, f1 = flat[h0], flat[h1]
            rows, cols, ok = _pack_tail_core(h0, h1, f0, f1)
            ok_all &= ok
            if not ok:
                break
            rowpad = (N_ATOMS + np.arange(128, dtype=np.int32))[:, None]
            p0i = np.broadcast_to(rowpad, (128, C)).astype(np.int32).copy()
            p1i = p0i.copy()
            # pad ids N_ATOMS+p: never equal a real atom; within a column all distinct
            a0 = p0i.astype(np.int64).copy()
            a1 = a0.copy()
            g0 = np.broadcast_to(NBINS + np.arange(128)[:, None],
                                 (128, C)).astype(np.int64).copy()
            g1 = g0.copy()
            a0[rows, cols] = h0
            a1[rows, cols] = h1
            g0[rows, cols] = f0
            g1[rows, cols] = f1
            p0i[rows, cols] = h0
            p1i[rows, cols] = h1
            ma0 = np.zeros((128, C, 128), np.float32)
            ma1 = np.zeros((128, C, 128), np.float32)
            mr0 = np.zeros((128, C, 128), np.float32)
            mr1 = np.zeros((128, C, 128), np.float32)
            for cc in range(C):
                A0, A1 = a0[:, cc], a1[:, cc]
                G0, G1 = g0[:, cc], g1[:, cc]
                ma0[:, cc, :] = ((A0[:, None] == A0[None, :]) +
                                 (A1[:, None] == A0[None, :])).astype(np.float32)
                ma1[:, cc, :] = ((A0[:, None] == A1[None, :]) +
                                 (A1[:, None] == A1[None, :])).astype(np.float32)
                mr0[:, cc, :] = ((G0[:, None] == G0[None, :]) +
                                 (G1[:, None] == G0[None, :])).astype(np.float32)
                mr1[:, cc, :] = ((G0[:, None] == G1[None, :]) +
                                 (G1[:, None] == G1[None, :])).astype(np.float32)
            in_maps.append(dict(
                co=co_in, adf=adf_in, alb=alb_in,
                p0i=p0i, p1i=p1i, p0f=p0i.astype(np.float32),
                ma0=ma0.reshape(128, -1), ma1=ma1.reshape(128, -1),
                mr0=mr0.reshape(128, -1), mr1=mr1.reshape(128, -1),
                prow=(NBINS + np.arange(128, dtype=np.float32))[:, None]))
        if ok_all:
            rest = bass_utils.run_bass_kernel_spmd(
                nct, in_maps, core_ids=list(range(N_CORES)))
            atomEnergy = np.zeros((N_ATOMS, A), np.float32)
            residueEnergy = np.zeros((NBINS, A), np.float32)
            for c in range(N_CORES):
                atomEnergy += rest.results[c]["ae"][:N_ATOMS]
                residueEnergy += rest.results[c]["re"][:NBINS]
        else:
            use_device_tail = False

    if not use_device_tail:
        atomEnergy = _host_tail(coords, resnum, alt, p0, p1, hit)
        residueEnergy = np.zeros((NBINS, A), np.float32)
        nz = np.flatnonzero(np.any(atomEnergy != 0, axis=1))
        np.add.at(residueEnergy, flat[nz], atomEnergy[nz])

    residueEnergy = residueEnergy.reshape(N_BATCH, N_CHAIN, N_RES, A)
    return residueEnergy, atomEnergy, sulfur


# revision 21
# speedup vs baseline: 2.6842x; 1.4474x over previous
"""Trainium2 Bass kernel for nn_Disulfide_net (edge-parallel GNN scatter).

Strategy (8 NeuronCores, SPMD):
  - Host shards atomPairs by sorting each endpoint column and grouping pairs
    into "chunks" of <=F pairs whose endpoint atoms span a <=31-atom window
    (graph-partition sharding + per-chunk halo of at_name bytes).
  - Device phase 1 (p1-sorted layout): per chunk, pack isSG (at_name==16) of
    the 31-atom window into an int32 bitmap held as a per-partition scalar;
    each pair tests its bit by ANDing the host-supplied 2^(p%31) selector
    against the bitmap -> m1 mask for all 8M pairs in 2 fused DVE passes
    (~1/64 cycle per pair).
  - Host relays m1 into the p0-sorted layout (pure index permutation).
  - Device phase 2 (p0-sorted layout): m0 bit-test AND m1 -> sulfur mask.
  - Device phase 3 (sparse tail, ~1/1024 of pairs): indirect-DMA gathers of
    per-atom records, dense energy evaluation, duplicate-group totals via
    per-column PE matmuls (host supplies 0/1 group matrices from pure index
    data), then indirect-DMA row writes into per-core atomEnergy /
    residueEnergy partials; host sums the 8 partials on unshard.
"""
import sys
import numpy as np

sys.path.insert(0, "/opt/trn_rl_repo")

N_CORES = 8
N_ATOMS = 500_000
N_PAIRS = 8_000_000
A = 4
N_BATCH, N_CHAIN, N_RES = 2, 4, 50000
SG_HASH = 16
TEMPERATURE = 298.0
WIN = 31          # atoms per chunk window (bit 31 of int32 never used)
F = 576           # pair slots per chunk (~16% pad at lambda~496)
_prog_cache = {}


def _build_layout(pvals, T):
    """Assign each pair a slot in the [8, 128, T, F] chunked layout, sorted by
    its endpoint atom id. Returns (slot_of_pair, per-core arrays)."""
    order = np.argsort(pvals, kind="stable")
    ps = pvals[order]
    win = ps // WIN
    uw, first = np.unique(win, return_index=True)
    cnts = np.diff(np.r_[first, len(ps)])
    nchunk_per_win = (cnts + F - 1) // F
    chunk_base = np.r_[0, np.cumsum(nchunk_per_win)][:-1]
    n_chunks = int(chunk_base[-1] + nchunk_per_win[-1]) if len(uw) else 0
    assert n_chunks <= 8 * 128 * T, f"{n_chunks} chunks > capacity {8*128*T}"
    rank = np.arange(len(ps)) - np.repeat(first, cnts)
    chunk_of = np.repeat(chunk_base, cnts) + rank // F
    slot_in = rank % F
    # chunk c -> core = c // (128*T), p = (c % (128*T)) % 128, t = (...) // 128
    c_loc = chunk_of % (128 * T)
    # [core, p, t, f] flat = ((core*128 + p)*T + t)*F + f
    slot = ((chunk_of // (128 * T)) * 128 + (c_loc % 128)) * (T * F) \
        + (c_loc // 128) * F + slot_in
    slot_of_pair = np.empty(len(ps), np.int64)
    slot_of_pair[order] = slot
    # per-chunk metadata, scattered into the [core, p, t] layout order
    cid = np.arange(n_chunks)
    cl = cid % (128 * T)
    lay = ((cid // (128 * T)) * 128 + (cl % 128)) * T + (cl // 128)
    base_of_chunk = np.zeros(8 * 128 * T, np.int64)
    base_of_chunk[lay] = np.repeat(uw, nchunk_per_win) * WIN
    # host ships the bit-select value 2^(p mod WIN) directly (pure layout
    # data); pad slots keep 0 -> mask 0 with no further handling
    pw = np.zeros((8, 128, T, F), np.int32)
    pw.reshape(-1)[slot] = (np.int32(1) << (ps % WIN).astype(np.int32))
    return slot_of_pair, pw, base_of_chunk.reshape(8, 128, T), n_chunks


def _build_anw(at_name_u8, base_of_chunk):
    """Per-chunk 32-byte at_name halo windows. [8,128,T,32] int8."""
    idx = base_of_chunk[..., None] + np.arange(32)
    valid = idx < N_ATOMS
    idxc = np.clip(idx, 0, N_ATOMS - 1)
    anw = at_name_u8[idxc]
    anw[~valid] = 0
    anw[..., 31] = 0  # bit31 unused
    return anw.astype(np.int8)


def _build_program(T, phase):
    import concourse.bacc as bacc
    import concourse.mybir as mybir
    from concourse.tile import TileContext
    dt = mybir.dt
    AluOp = mybir.AluOpType

    nc = bacc.Bacc("TRN2", target_bir_lowering=False, debug=False,
                   num_devices=N_CORES)
    TF = T * F
    pw_d = nc.dram_tensor("pw", [128, TF], dt.int32, kind="ExternalInput")
    anw_d = nc.dram_tensor("anw", [128, T * 32], dt.int8, kind="ExternalInput")
    wts_d = nc.dram_tensor("wts", [128, T * 32], dt.float32, kind="ExternalInput")
    if phase == 2:
        mfac_d = nc.dram_tensor("mfac", [128, TF], dt.int8, kind="ExternalInput")
    mby_d = nc.dram_tensor("mby", [128, TF], dt.int8, kind="ExternalOutput")

    with TileContext(nc) as tc:
        with tc.tile_pool(name="big", bufs=1) as big, \
             tc.tile_pool(name="small", bufs=3) as small:
            pwt = big.tile([128, TF], dt.int32)
            mby = big.tile([128, TF], dt.int8)
            anw = big.tile([128, T * 32], dt.int8)
            wts = big.tile([128, T * 32], dt.float32)
            nc.sync.dma_start(pwt[:], pw_d.ap())
            nc.sync.dma_start(anw[:], anw_d.ap())
            nc.sync.dma_start(wts[:], wts_d.ap())
            if phase == 2:
                mfac = big.tile([128, TF], dt.int8)
                nc.sync.dma_start(mfac[:], mfac_d.ap())

            # ---- bitmap build: B[p, t] int32 of isSG over the chunk window
            anf = big.tile([128, T * 32], dt.float32)
            nc.vector.tensor_copy(anf[:], anw[:])
            eq = big.tile([128, T * 32], dt.float32)
            nc.vector.tensor_scalar(eq[:], anf[:], float(SG_HASH), None,
                                    AluOp.is_equal)
            nc.vector.tensor_tensor(eq[:], eq[:], wts[:], AluOp.mult)
            eq3 = eq[:].rearrange("p (t k) -> p t k", k=32)
            lo = big.tile([128, T], dt.float32)
            hi = big.tile([128, T], dt.float32)
            nc.vector.tensor_reduce(lo[:], eq3[:, :, 0:16],
                                    axis=mybir.AxisListType.X, op=AluOp.add)
            nc.vector.tensor_reduce(hi[:], eq3[:, :, 16:32],
                                    axis=mybir.AxisListType.X, op=AluOp.add)
            loi = big.tile([128, T], dt.int32)
            hii = big.tile([128, T], dt.int32)
            nc.vector.tensor_copy(loi[:], lo[:])
            nc.vector.tensor_copy(hii[:], hi[:])
            nc.vector.tensor_scalar(hii[:], hii[:], 16, None,
                                    AluOp.logical_shift_left)
            B = big.tile([128, T], dt.int32)
            nc.vector.tensor_tensor(B[:], loi[:], hii[:], AluOp.bitwise_or)

            # ---- per-tile mask: 2 fused passes (2^w supplied by host)
            for t in range(T):
                sl = slice(t * F, (t + 1) * F)
                mr = small.tile([128, F], dt.int32, tag="mr")
                nc.vector.tensor_scalar(mr[:], pwt[:, sl], B[:, t:t + 1], None,
                                        AluOp.bitwise_and)
                if phase == 2:
                    # (bit != 0) AND m1 {0,1} int8 relay
                    nc.vector.scalar_tensor_tensor(
                        mby[:, sl], mr[:], 0, mfac[:, sl],
                        AluOp.not_equal, AluOp.logical_and)
                else:
                    nc.vector.tensor_scalar(mby[:, sl], mr[:], 0, None,
                                            AluOp.not_equal)
            nc.sync.dma_start(mby_d.ap(), mby[:])
    nc.compile()
    return nc

def _get_program(T, phase):
    key = (T, phase)
    if key not in _prog_cache:
        _prog_cache[key] = _build_program(T, phase)
    return _prog_cache[key]


def _run_phase(nc, pw, anw, wts, mfac=None):
    from concourse import bass_utils
    in_maps = []
    for c in range(N_CORES):
        m = dict(pw=np.ascontiguousarray(pw[c].reshape(128, -1)),
                 anw=np.ascontiguousarray(anw[c].reshape(128, -1)),
                 wts=wts)
        if mfac is not None:
            m["mfac"] = np.ascontiguousarray(mfac[c].reshape(128, -1))
        in_maps.append(m)
    res = bass_utils.run_bass_kernel_spmd(nc, in_maps,
                                          core_ids=list(range(N_CORES)))
    return res


C_PAIR = 8            # pair columns per core in the tail (capacity 1024 pairs)
REC_P = 3936          # 128*3936 = 503808 >= N_ATOMS
NBINS = N_BATCH * N_CHAIN * N_RES


def _build_tail_program():
    import concourse.bacc as bacc
    import concourse.mybir as mybir
    from concourse.tile import TileContext
    dt = mybir.dt
    AluOp = mybir.AluOpType
    AF = mybir.ActivationFunctionType
    from concourse.bass import IndirectOffsetOnAxis

    C = C_PAIR
    nc = bacc.Bacc("TRN2", target_bir_lowering=False, debug=False,
                   num_devices=N_CORES)
    rec_t = nc.dram_tensor("rect", [128 * REC_P, 8], dt.float32,
                           kind="ExternalInput")
    p0i_d = nc.dram_tensor("p0i", [128, C], dt.int32, kind="ExternalInput")
    p1i_d = nc.dram_tensor("p1i", [128, C], dt.int32, kind="ExternalInput")
    p0f_d = nc.dram_tensor("p0f", [128, C], dt.float32, kind="ExternalInput")
    ma0_d = nc.dram_tensor("ma0", [128, C * 128], dt.float32, kind="ExternalInput")
    ma1_d = nc.dram_tensor("ma1", [128, C * 128], dt.float32, kind="ExternalInput")
    mr0_d = nc.dram_tensor("mr0", [128, C * 128], dt.float32, kind="ExternalInput")
    mr1_d = nc.dram_tensor("mr1", [128, C * 128], dt.float32, kind="ExternalInput")
    prow_d = nc.dram_tensor("prow", [128, 1], dt.float32, kind="ExternalInput")
    ae_d = nc.dram_tensor("ae", [N_ATOMS + 128, A], dt.float32, kind="ExternalOutput")
    re_d = nc.dram_tensor("re", [NBINS + 128, A], dt.float32, kind="ExternalOutput")
    dbg_d = nc.dram_tensor("dbg", [128, C * 24], dt.float32, kind="ExternalOutput")
    dbg2_d = nc.dram_tensor("dbg2", [1, 128], dt.float32, kind="ExternalOutput")

    with TileContext(nc) as tc:
        with tc.tile_pool(name="big", bufs=1) as big, \
             tc.tile_pool(name="sm", bufs=2) as sm, \
             tc.tile_pool(name="ps", bufs=2, space="PSUM") as ps:
            rec_dram = rec_t
            p0i = big.tile([128, C], dt.int32)
            p1i = big.tile([128, C], dt.int32)
            p0f = big.tile([128, C], dt.float32)
            prow_d_col = big.tile([128, 1], dt.float32)
            nc.sync.dma_start(p0i[:], p0i_d.ap())
            nc.sync.dma_start(p1i[:], p1i_d.ap())
            nc.sync.dma_start(p0f[:], p0f_d.ap())
            nc.sync.dma_start(prow_d_col[:], prow_d.ap())
            rec0f = big.tile([128, C * 8], dt.float32)
            rec1f = big.tile([128, C * 8], dt.float32)
            nc.gpsimd.memset(rec0f[:], 0.0)
            nc.gpsimd.memset(rec1f[:], 0.0)
            for c in range(C):
                nc.gpsimd.indirect_dma_start(
                    rec0f[:, c * 8:(c + 1) * 8], None, rec_dram.ap(),
                    IndirectOffsetOnAxis(ap=p0i[:, c:c + 1], axis=0))
                nc.gpsimd.indirect_dma_start(
                    rec1f[:, c * 8:(c + 1) * 8], None, rec_dram.ap(),
                    IndirectOffsetOnAxis(ap=p1i[:, c:c + 1], axis=0))
            rec0 = rec0f[:].rearrange("p (c e) -> p c e", e=8)
            rec1 = rec1f[:].rearrange("p (c e) -> p c e", e=8)

            # ---- energy per pair slot [128, C]
            d2 = big.tile([128, C], dt.float32)
            tmp = big.tile([128, C], dt.float32)
            nc.gpsimd.memset(d2[:], 0.0)
            for k in range(3):
                dx = sm.tile([128, C], dt.float32, tag="dx")
                nc.vector.tensor_tensor(dx[:], rec0[:, :, k], rec1[:, :, k],
                                        AluOp.subtract)
                nc.vector.tensor_scalar(dx[:], dx[:], 1e-6, None, AluOp.add)
                nc.vector.tensor_tensor(dx[:], dx[:], dx[:], AluOp.mult)
                nc.vector.tensor_tensor(d2[:], d2[:], dx[:], AluOp.add)
            dist = big.tile([128, C], dt.float32)
            nc.scalar.activation(dist[:], d2[:], mybir.ActivationFunctionType.Sqrt)
            padv01 = big.tile([128, C], dt.float32)
            nc.vector.tensor_scalar(padv01[:], p0f[:], float(N_ATOMS) - 0.5,
                                    None, AluOp.is_gt)
            rd = big.tile([128, C], dt.float32)
            nc.vector.tensor_tensor(rd[:], rec0[:, :, 3], rec1[:, :, 3],
                                    AluOp.subtract)
            nc.scalar.activation(rd[:], rd[:], mybir.ActivationFunctionType.Abs)
            nc.vector.tensor_tensor(rd[:], rd[:], padv01[:], AluOp.add)
            lg = big.tile([128, C], dt.float32)
            nc.scalar.activation(lg[:], rd[:], mybir.ActivationFunctionType.Ln)
            netE = big.tile([128, C], dt.float32)
            # netE = 0.5*(-0.298*(2.1 + 2.9823825*lg) + 5*|dist-2.04|)
            nc.vector.tensor_scalar(netE[:], lg[:],
                                    0.5 * -0.001 * TEMPERATURE * 2.9823825,
                                    0.5 * -0.001 * TEMPERATURE * 2.1,
                                    AluOp.mult, AluOp.add)
            nc.vector.tensor_scalar(tmp[:], dist[:], 2.04, None,
                                    AluOp.subtract)
            nc.scalar.activation(tmp[:], tmp[:], mybir.ActivationFunctionType.Abs)
            nc.vector.tensor_scalar(tmp[:], tmp[:], 2.5, None, AluOp.mult)
            nc.vector.tensor_tensor(netE[:], netE[:], tmp[:], AluOp.add)
            # alt bits -> contrib [128, C, 4]
            ab = big.tile([128, C], dt.int32)
            a0i = big.tile([128, C], dt.int32)
            a1i = big.tile([128, C], dt.int32)
            nc.vector.tensor_copy(a0i[:], rec0[:, :, 6])
            nc.vector.tensor_copy(a1i[:], rec1[:, :, 6])
            nc.vector.tensor_tensor(ab[:], a0i[:], a1i[:], AluOp.bitwise_and)
            contrib = big.tile([128, C, 4], dt.float32)
            for k in range(A):
                bk = sm.tile([128, C], dt.int32, tag="bk")
                bf = sm.tile([128, C], dt.float32, tag="bf")
                nc.vector.tensor_scalar(bk[:], ab[:], 1 << k, None,
                                        AluOp.bitwise_and)
                nc.vector.tensor_copy(bf[:], bk[:])
                nc.vector.tensor_scalar(bf[:], bf[:], 0.0, None, AluOp.not_equal)
                nc.vector.tensor_tensor(contrib[:, :, k], netE[:], bf[:],
                                        AluOp.mult)

            # ---- flat offsets (+pad fix: pads have p0f > N_ATOMS)
            # pads write to dump rows NBINS+p (sliced off on host)
            prow = big.tile([128, C], dt.float32)
            nc.vector.tensor_scalar(prow[:], padv01[:], prow_d_col[:], None,
                                    AluOp.mult)
            vm = big.tile([128, C], dt.float32)
            nc.vector.tensor_scalar(vm[:], padv01[:], -1.0, 1.0, AluOp.mult,
                                    AluOp.add)
            # flat = b*200000 + c*50000 + resnum, from gathered b/c/r
            f0 = big.tile([128, C], dt.float32)
            f1 = big.tile([128, C], dt.float32)
            fb = big.tile([128, C], dt.float32)
            for (ft, rc) in ((f0, rec0), (f1, rec1)):
                nc.vector.tensor_scalar(ft[:], rc[:, :, 4], 200000.0, None,
                                        AluOp.mult)
                nc.vector.tensor_scalar(fb[:], rc[:, :, 5], 50000.0, None,
                                        AluOp.mult)
                nc.vector.tensor_tensor(ft[:], ft[:], fb[:], AluOp.add)
                nc.vector.tensor_tensor(ft[:], ft[:], rc[:, :, 3], AluOp.add)
                nc.vector.tensor_tensor(ft[:], ft[:], vm[:], AluOp.mult)
                nc.vector.tensor_tensor(ft[:], ft[:], prow[:], AluOp.add)
            f0i = big.tile([128, C], dt.int32)
            f1i = big.tile([128, C], dt.int32)
            nc.vector.tensor_copy(f0i[:], f0[:])
            nc.vector.tensor_copy(f1i[:], f1[:])

            # ---- dedup totals via per-column group matmuls, then write scatters
            ma0 = big.tile([128, C * 128], dt.float32)
            ma1 = big.tile([128, C * 128], dt.float32)
            mr0 = big.tile([128, C * 128], dt.float32)
            mr1 = big.tile([128, C * 128], dt.float32)
            nc.sync.dma_start(ma0[:], ma0_d.ap())
            nc.sync.dma_start(ma1[:], ma1_d.ap())
            nc.sync.dma_start(mr0[:], mr0_d.ap())
            nc.sync.dma_start(mr1[:], mr1_d.ap())
            dbg = big.tile([128, C * 24], dt.float32)
            nc.vector.tensor_copy(dbg[:].rearrange("p (c e) -> p c e", e=24)[:, :, 0:8], rec0)
            nc.vector.tensor_copy(dbg[:].rearrange("p (c e) -> p c e", e=24)[:, :, 8:16], rec1)
            nc.vector.tensor_copy(dbg[:].rearrange("p (c e) -> p c e", e=24)[:, :, 16:17], netE[:].rearrange("p c -> p c ()"))
            nc.vector.tensor_copy(dbg[:].rearrange("p (c e) -> p c e", e=24)[:, :, 17:21], contrib[:])
            nc.vector.tensor_copy(dbg[:].rearrange("p (c e) -> p c e", e=24)[:, :, 21:22], rd[:].rearrange("p c -> p c ()"))
            nc.vector.tensor_copy(dbg[:].rearrange("p (c e) -> p c e", e=24)[:, :, 22:23], dist[:].rearrange("p c -> p c ()"))
            nc.sync.dma_start(dbg_d.ap(), dbg[:])
            tots = {}
            for name, m in (("a0", ma0), ("a1", ma1), ("r0", mr0), ("r1", mr1)):
                tot = big.tile([128, C * 4], dt.float32, tag="tot" + name)
                for c in range(C):
                    pt = ps.tile([128, 4], dt.float32, tag="pt")
                    nc.tensor.matmul(out=pt[:], lhsT=m[:, c * 128:(c + 1) * 128],
                                     rhs=contrib[:, c, :], start=True, stop=True)
                    nc.vector.tensor_copy(tot[:, c * 4:(c + 1) * 4], pt[:])
                tots[name] = tot
            for c in range(C):
                nc.gpsimd.indirect_dma_start(
                    ae_d.ap(), IndirectOffsetOnAxis(ap=p0i[:, c:c + 1], axis=0),
                    tots["a0"][:, c * 4:(c + 1) * 4], None)
                nc.gpsimd.indirect_dma_start(
                    ae_d.ap(), IndirectOffsetOnAxis(ap=p1i[:, c:c + 1], axis=0),
                    tots["a1"][:, c * 4:(c + 1) * 4], None)
                nc.gpsimd.indirect_dma_start(
                    re_d.ap(), IndirectOffsetOnAxis(ap=f0i[:, c:c + 1], axis=0),
                    tots["r0"][:, c * 4:(c + 1) * 4], None)
                nc.gpsimd.indirect_dma_start(
                    re_d.ap(), IndirectOffsetOnAxis(ap=f1i[:, c:c + 1], axis=0),
                    tots["r1"][:, c * 4:(c + 1) * 4], None)
    nc.compile()
    return nc


def _get_tail_program():
    if "tail" not in _prog_cache:
        _prog_cache["tail"] = _build_tail_program()
    return _prog_cache["tail"]


class _UF:
    def __init__(self, n):
        self.p = list(range(n))

    def find(self, x):
        while self.p[x] != x:
            self.p[x] = self.p[self.p[x]]
            x = self.p[x]
        return x

    def union(self, a, b):
        ra, rb = self.find(a), self.find(b)
        if ra != rb:
            self.p[ra] = rb


def _pack_tail_core(h0, h1, fl0, fl1):
    """Pack this core's hit pairs into [128, C_PAIR] columns such that no two
    columns share an atom id or flat id. Returns (rows, cols, ok)."""
    n = len(h0)
    C = C_PAIR
    if n == 0:
        return np.zeros(0, np.int64), np.zeros(0, np.int64), True
    uf = _UF(n)
    for keys in (np.concatenate([h0, h1]), np.concatenate([fl0, fl1])):
        pid = np.tile(np.arange(n), 2)
        o = np.argsort(keys, kind="stable")
        ks, po = keys[o], pid[o]
        same = np.flatnonzero(ks[1:] == ks[:-1])
        for i in same:
            uf.union(int(po[i]), int(po[i + 1]))
    root = np.array([uf.find(i) for i in range(n)])
    comps = {}
    for i, r in enumerate(root):
        comps.setdefault(r, []).append(i)
    # first-fit into C columns of capacity 128
    cap = [128] * C
    rows = np.empty(n, np.int64)
    cols = np.empty(n, np.int64)
    for members in sorted(comps.values(), key=len, reverse=True):
        sz = len(members)
        if sz > 128:
            return rows, cols, False
        placed = False
        for c in range(C):
            if cap[c] >= sz:
                r0 = 128 - cap[c]
                for j, m in enumerate(members):
                    rows[m] = r0 + j
                    cols[m] = c
                cap[c] -= sz
                placed = True
                break
        if not placed:
            return rows, cols, False
    return rows, cols, True


def _host_tail(coords, resnum, alt, p0, p1, hit):
    atomEnergy = np.zeros((N_ATOMS, A), np.float32)
    if len(hit):
        h0, h1 = p0[hit], p1[hit]
        diff = coords[h0] - coords[h1] + 1e-6
        dist = np.sqrt(np.sum(diff * diff, axis=-1))
        rd = np.abs(resnum[h0] - resnum[h1]).astype(np.float32)
        energy = -0.001 * TEMPERATURE * (2.1 + 2.9823825 * np.log(rd)) \
            + 5.0 * np.abs(dist - 2.04)
        netE = 0.5 * energy
        pair_alt = alt[h0] & alt[h1]
        contrib = np.where(pair_alt, netE[:, None], 0.0).astype(np.float32)
        np.add.at(atomEnergy, h0, contrib)
        np.add.at(atomEnergy, h1, contrib)
    return atomEnergy


def kernel(coords, atom_description, atom_number, atomPairs, alternativeMask,
           partners, facc):
    coords = np.asarray(coords, np.float32)
    ad = np.asarray(atom_description)
    pairs = np.asarray(atomPairs).astype(np.int64)
    alt = np.asarray(alternativeMask).astype(bool)
    at_name = ad[:, 3].astype(np.int64)
    resnum = ad[:, 2].astype(np.int64)
    p0, p1 = pairs[:, 0], pairs[:, 1]
    npair = len(p0)

    at_u8 = at_name.astype(np.uint8)

    # choose T adaptively (worst-case chunk count over both sides)
    def n_chunks_needed(p):
        win_counts = np.bincount(p // WIN)
        wc = win_counts[win_counts > 0]
        return int(np.sum((wc + F - 1) // F))

    need = max(n_chunks_needed(p0), n_chunks_needed(p1))
    T = max(1, -(-need // (8 * 128)))
    nc1 = _get_program(T, 1)
    nc2 = _get_program(T, 2)

    # weights for bitmap packing
    k = np.arange(32)
    w_lo = np.where(k < 16, 2.0 ** k, 0.0)
    w_hi = np.where(k >= 16, 2.0 ** (k - 16), 0.0)
    wts = np.tile((w_lo + w_hi).astype(np.float32), T)[None, :].repeat(128, 0)
    wts[:, 31::32] = 0.0

    # ---- phase 1: p1 side
    slot1, pw1, boc1, _ = _build_layout(p1, T)
    anw1 = _build_anw(at_u8, boc1)
    res1 = _run_phase(nc1, pw1, anw1, wts)
    m1 = np.stack([res1.results[c]["mby"] for c in range(N_CORES)])  # [8,128,TF]
    m1_of_pair = m1.reshape(-1)[slot1]

    # ---- relay into p0 layout ({0,1} int8; phase 2 fuses ne + logical_and)
    slot0, pw0, boc0, _ = _build_layout(p0, T)
    anw0 = _build_anw(at_u8, boc0)
    mfac0 = np.zeros(8 * 128 * T * F, np.int8)
    mfac0[slot0] = m1_of_pair
    res0 = _run_phase(nc2, pw0, anw0, wts,
                      mfac0.reshape(8, 128, T * F))
    sby = np.stack([res0.results[c]["mby"] for c in range(N_CORES)])
    sulfur = sby.reshape(-1)[slot0] != 0

    # ---- sparse tail on device (phase 3); host fallback for over-capacity
    flat = ((ad[:, 0].astype(np.int64) * N_CHAIN + ad[:, 1]) * N_RES + resnum)
    hit = np.flatnonzero(sulfur)
    # Device tail by default; host fallback only for over-capacity inputs
    # (>128*C_PAIR sulfur pairs per core) or if explicitly disabled.
    import os
    use_device_tail = (os.environ.get("DISULF_DEVICE_TAIL", "1") == "1"
                       and len(hit) <= N_CORES * 128 * C_PAIR)

    if use_device_tail:
        from concourse import bass_utils
        nct = _get_tail_program()
        C = C_PAIR
        # host-assembled record table (pure relayout of input columns):
        # [x, y, z, resnum, batch, chain, altbits, 0]
        NP_ = 128 * REC_P
        rec_in = np.zeros((NP_, 8), np.float32)
        rec_in[:N_ATOMS, 0:3] = coords
        rec_in[:N_ATOMS, 3] = resnum
        rec_in[:N_ATOMS, 4] = ad[:, 0]
        rec_in[:N_ATOMS, 5] = ad[:, 1]
        rec_in[:N_ATOMS, 6] = (alt * (1 << np.arange(4))).sum(1)
        # split hits evenly across cores
        in_maps = []
        per = [hit[c::N_CORES] for c in range(N_CORES)]
        ok_all = True
        for c in range(N_CORES):
            hc = per[c]
            h0, h1 = p0[hc], p1[hc]
            f0, f1 = flat[h0], flat[h1]
            rows, cols, ok = _pack_tail_core(h0, h1, f0, f1)
            ok_all &= ok
            if not ok:
                break
            rowpad = (N_ATOMS + np.arange(128, dtype=np.int32))[:, None]
            p0i = np.broadcast_to(rowpad, (128, C)).astype(np.int32).copy()
            p1i = p0i.copy()
            # pad ids N_ATOMS+p: never equal a real atom; within a column all distinct
            a0 = p0i.astype(np.int64).copy()
            a1 = a0.copy()
            g0 = np.broadcast_to(NBINS + np.arange(128)[:, None],
                                 (128, C)).astype(np.int64).copy()
            g1 = g0.copy()
            a0[rows, cols] = h0
            a1[rows, cols] = h1
            g0[rows, cols] = f0
            g1[rows, cols] = f1
            p0i[rows, cols] = h0
            p1i[rows, cols] = h1
            ma0 = np.zeros((128, C, 128), np.float32)
            ma1 = np.zeros((128, C, 128), np.float32)
            mr0 = np.zeros((128, C, 128), np.float32)
            mr1 = np.zeros((128, C, 128), np.float32)
            for cc in range(C):
                A0, A1 = a0[:, cc], a1[:, cc]
                G0, G1 = g0[:, cc], g1[:, cc]
                ma0[:, cc, :] = ((A0[:, None] == A0[None, :]) +
                                 (A1[:, None] == A0[None, :])).astype(np.float32)
                ma1[:, cc, :] = ((A0[:, None] == A1[None, :]) +
                                 (A1[:, None] == A1[None, :])).astype(np.float32)
                mr0[:, cc, :] = ((G0[:, None] == G0[None, :]) +
                                 (G1[:, None] == G0[None, :])).astype(np.float32)
                mr1[:, cc, :] = ((G0[:, None] == G1[None, :]) +
                                 (G1[:, None] == G1[None, :])).astype(np.float32)
            in_maps.append(dict(
                rect=rec_in,
                p0i=p0i, p1i=p1i, p0f=p0i.astype(np.float32),
                ma0=ma0.reshape(128, -1), ma1=ma1.reshape(128, -1),
                mr0=mr0.reshape(128, -1), mr1=mr1.reshape(128, -1),
                prow=(NBINS + np.arange(128, dtype=np.float32))[:, None]))
        if ok_all:
            rest = bass_utils.run_bass_kernel_spmd(
                nct, in_maps, core_ids=list(range(N_CORES)))
            atomEnergy = np.zeros((N_ATOMS, A), np.float32)
            residueEnergy = np.zeros((NBINS, A), np.float32)
            for c in range(N_CORES):
                atomEnergy += rest.results[c]["ae"][:N_ATOMS]
                residueEnergy += rest.results[c]["re"][:NBINS]
        else:
            use_device_tail = False

    if not use_device_tail:
        atomEnergy = _host_tail(coords, resnum, alt, p0, p1, hit)
        residueEnergy = np.zeros((NBINS, A), np.float32)
        nz = np.flatnonzero(np.any(atomEnergy != 0, axis=1))
        np.add.at(residueEnergy, flat[nz], atomEnergy[nz])

    residueEnergy = residueEnergy.reshape(N_BATCH, N_CHAIN, N_RES, A)
    return residueEnergy, atomEnergy, sulfur


# revision 22
# speedup vs baseline: 2.9143x; 1.0857x over previous
"""Trainium2 Bass kernel for nn_Disulfide_net (edge-parallel GNN scatter).

Strategy (8 NeuronCores, SPMD):
  - Host shards atomPairs by sorting each endpoint column and grouping pairs
    into "chunks" of <=F pairs whose endpoint atoms span a <=31-atom window
    (graph-partition sharding + per-chunk halo of at_name bytes).
  - Device phase 1 (p1-sorted layout): per chunk, pack isSG (at_name==16) of
    the 31-atom window into an int32 bitmap held as a per-partition scalar;
    each pair tests its bit by ANDing the host-supplied 2^(p%31) selector
    against the bitmap -> m1 mask for all 8M pairs in 2 fused DVE passes
    (~1/64 cycle per pair).
  - Host relays m1 into the p0-sorted layout (pure index permutation).
  - Device phase 2 (p0-sorted layout): m0 bit-test AND m1 -> sulfur mask.
  - Device phase 3 (sparse tail, ~1/1024 of pairs): indirect-DMA gathers of
    per-atom records, dense energy evaluation, duplicate-group totals via
    per-column PE matmuls (host supplies 0/1 group matrices from pure index
    data), then indirect-DMA row writes into per-core atomEnergy /
    residueEnergy partials; host sums the 8 partials on unshard.
"""
import sys
import numpy as np

sys.path.insert(0, "/opt/trn_rl_repo")

N_CORES = 8
N_ATOMS = 500_000
N_PAIRS = 8_000_000
A = 4
N_BATCH, N_CHAIN, N_RES = 2, 4, 50000
SG_HASH = 16
TEMPERATURE = 298.0
WIN = 31          # atoms per chunk window (bit 31 of int32 never used)
F = 576           # pair slots per chunk (~16% pad at lambda~496)
_prog_cache = {}


def _build_layout(pvals, T):
    """Assign each pair a slot in the [8, 128, T, F] chunked layout, sorted by
    its endpoint atom id. Returns (slot_of_pair, per-core arrays)."""
    order = np.argsort(pvals, kind="stable")
    ps = pvals[order]
    win = ps // WIN
    uw, first = np.unique(win, return_index=True)
    cnts = np.diff(np.r_[first, len(ps)])
    nchunk_per_win = (cnts + F - 1) // F
    chunk_base = np.r_[0, np.cumsum(nchunk_per_win)][:-1]
    n_chunks = int(chunk_base[-1] + nchunk_per_win[-1]) if len(uw) else 0
    assert n_chunks <= 8 * 128 * T, f"{n_chunks} chunks > capacity {8*128*T}"
    rank = np.arange(len(ps)) - np.repeat(first, cnts)
    chunk_of = np.repeat(chunk_base, cnts) + rank // F
    slot_in = rank % F
    # chunk c -> core = c // (128*T), p = (c % (128*T)) % 128, t = (...) // 128
    c_loc = chunk_of % (128 * T)
    # [core, p, t, f] flat = ((core*128 + p)*T + t)*F + f
    slot = ((chunk_of // (128 * T)) * 128 + (c_loc % 128)) * (T * F) \
        + (c_loc // 128) * F + slot_in
    slot_of_pair = np.empty(len(ps), np.int64)
    slot_of_pair[order] = slot
    # per-chunk metadata, scattered into the [core, p, t] layout order
    cid = np.arange(n_chunks)
    cl = cid % (128 * T)
    lay = ((cid // (128 * T)) * 128 + (cl % 128)) * T + (cl // 128)
    base_of_chunk = np.zeros(8 * 128 * T, np.int64)
    base_of_chunk[lay] = np.repeat(uw, nchunk_per_win) * WIN
    # host ships the bit-select value 2^(p mod WIN) directly (pure layout
    # data); pad slots keep 0 -> mask 0 with no further handling
    pw = np.zeros((8, 128, T, F), np.int32)
    pw.reshape(-1)[slot] = (np.int32(1) << (ps % WIN).astype(np.int32))
    return slot_of_pair, pw, base_of_chunk.reshape(8, 128, T), n_chunks


def _build_anw(at_name_u8, base_of_chunk):
    """Per-chunk 32-byte at_name halo windows. [8,128,T,32] int8."""
    idx = base_of_chunk[..., None] + np.arange(32)
    valid = idx < N_ATOMS
    idxc = np.clip(idx, 0, N_ATOMS - 1)
    anw = at_name_u8[idxc]
    anw[~valid] = 0
    anw[..., 31] = 0  # bit31 unused
    return anw.astype(np.int8)


def _build_program(T, phase):
    import concourse.bacc as bacc
    import concourse.mybir as mybir
    from concourse.tile import TileContext
    dt = mybir.dt
    AluOp = mybir.AluOpType

    nc = bacc.Bacc("TRN2", target_bir_lowering=False, debug=False,
                   num_devices=N_CORES)
    TF = T * F
    pw_d = nc.dram_tensor("pw", [128, TF], dt.int32, kind="ExternalInput")
    anw_d = nc.dram_tensor("anw", [128, T * 32], dt.int8, kind="ExternalInput")
    wts_d = nc.dram_tensor("wts", [128, T * 32], dt.float32, kind="ExternalInput")
    if phase == 2:
        mfac_d = nc.dram_tensor("mfac", [128, TF], dt.int8, kind="ExternalInput")
    mby_d = nc.dram_tensor("mby", [128, TF], dt.int8, kind="ExternalOutput")

    with TileContext(nc) as tc:
        with tc.tile_pool(name="big", bufs=1) as big, \
             tc.tile_pool(name="small", bufs=3) as small:
            pwt = big.tile([128, TF], dt.int32)
            mby = big.tile([128, TF], dt.int8)
            anw = big.tile([128, T * 32], dt.int8)
            wts = big.tile([128, T * 32], dt.float32)
            nc.sync.dma_start(pwt[:], pw_d.ap())
            nc.sync.dma_start(anw[:], anw_d.ap())
            nc.sync.dma_start(wts[:], wts_d.ap())
            if phase == 2:
                mfac = big.tile([128, TF], dt.int8)
                nc.sync.dma_start(mfac[:], mfac_d.ap())

            # ---- bitmap build: B[p, t] int32 of isSG over the chunk window
            anf = big.tile([128, T * 32], dt.float32)
            nc.vector.tensor_copy(anf[:], anw[:])
            eq = big.tile([128, T * 32], dt.float32)
            nc.vector.tensor_scalar(eq[:], anf[:], float(SG_HASH), None,
                                    AluOp.is_equal)
            nc.vector.tensor_tensor(eq[:], eq[:], wts[:], AluOp.mult)
            eq3 = eq[:].rearrange("p (t k) -> p t k", k=32)
            lo = big.tile([128, T], dt.float32)
            hi = big.tile([128, T], dt.float32)
            nc.vector.tensor_reduce(lo[:], eq3[:, :, 0:16],
                                    axis=mybir.AxisListType.X, op=AluOp.add)
            nc.vector.tensor_reduce(hi[:], eq3[:, :, 16:32],
                                    axis=mybir.AxisListType.X, op=AluOp.add)
            loi = big.tile([128, T], dt.int32)
            hii = big.tile([128, T], dt.int32)
            nc.vector.tensor_copy(loi[:], lo[:])
            nc.vector.tensor_copy(hii[:], hi[:])
            nc.vector.tensor_scalar(hii[:], hii[:], 16, None,
                                    AluOp.logical_shift_left)
            B = big.tile([128, T], dt.int32)
            nc.vector.tensor_tensor(B[:], loi[:], hii[:], AluOp.bitwise_or)

            # ---- per-tile mask: 2 fused passes (2^w supplied by host)
            for t in range(T):
                sl = slice(t * F, (t + 1) * F)
                mr = small.tile([128, F], dt.int32, tag="mr")
                nc.vector.tensor_scalar(mr[:], pwt[:, sl], B[:, t:t + 1], None,
                                        AluOp.bitwise_and)
                if phase == 2:
                    # (bit != 0) AND m1 {0,1} int8 relay
                    nc.vector.scalar_tensor_tensor(
                        mby[:, sl], mr[:], 0, mfac[:, sl],
                        AluOp.not_equal, AluOp.logical_and)
                else:
                    nc.vector.tensor_scalar(mby[:, sl], mr[:], 0, None,
                                            AluOp.not_equal)
            nc.sync.dma_start(mby_d.ap(), mby[:])
    nc.compile()
    return nc

def _get_program(T, phase):
    key = (T, phase)
    if key not in _prog_cache:
        _prog_cache[key] = _build_program(T, phase)
    return _prog_cache[key]


def _run_phase(nc, pw, anw, wts, mfac=None):
    from concourse import bass_utils
    in_maps = []
    for c in range(N_CORES):
        m = dict(pw=np.ascontiguousarray(pw[c].reshape(128, -1)),
                 anw=np.ascontiguousarray(anw[c].reshape(128, -1)),
                 wts=wts)
        if mfac is not None:
            m["mfac"] = np.ascontiguousarray(mfac[c].reshape(128, -1))
        in_maps.append(m)
    res = bass_utils.run_bass_kernel_spmd(nc, in_maps,
                                          core_ids=list(range(N_CORES)))
    return res


C_PAIR = 8            # pair columns per core in the tail (capacity 1024 pairs)
REC_P = 3936          # 128*3936 = 503808 >= N_ATOMS
NBINS = N_BATCH * N_CHAIN * N_RES


def _build_tail_program():
    import concourse.bacc as bacc
    import concourse.mybir as mybir
    from concourse.tile import TileContext
    dt = mybir.dt
    AluOp = mybir.AluOpType
    AF = mybir.ActivationFunctionType
    from concourse.bass import IndirectOffsetOnAxis

    C = C_PAIR
    nc = bacc.Bacc("TRN2", target_bir_lowering=False, debug=False,
                   num_devices=N_CORES)
    rec_t = nc.dram_tensor("rect", [128 * REC_P, 8], dt.float32,
                           kind="ExternalInput")
    p0i_d = nc.dram_tensor("p0i", [128, C], dt.int32, kind="ExternalInput")
    p1i_d = nc.dram_tensor("p1i", [128, C], dt.int32, kind="ExternalInput")
    p0f_d = nc.dram_tensor("p0f", [128, C], dt.float32, kind="ExternalInput")
    ma0_d = nc.dram_tensor("ma0", [128, C * 128], dt.float32, kind="ExternalInput")
    ma1_d = nc.dram_tensor("ma1", [128, C * 128], dt.float32, kind="ExternalInput")
    mr0_d = nc.dram_tensor("mr0", [128, C * 128], dt.float32, kind="ExternalInput")
    mr1_d = nc.dram_tensor("mr1", [128, C * 128], dt.float32, kind="ExternalInput")
    prow_d = nc.dram_tensor("prow", [128, 1], dt.float32, kind="ExternalInput")
    ae_d = nc.dram_tensor("ae", [N_ATOMS + 128, A], dt.float32, kind="ExternalOutput")
    re_d = nc.dram_tensor("re", [NBINS + 128, A], dt.float32, kind="ExternalOutput")
    dbg_d = nc.dram_tensor("dbg", [128, C * 24], dt.float32, kind="ExternalOutput")
    dbg2_d = nc.dram_tensor("dbg2", [1, 128], dt.float32, kind="ExternalOutput")

    with TileContext(nc) as tc:
        with tc.tile_pool(name="big", bufs=1) as big, \
             tc.tile_pool(name="sm", bufs=2) as sm, \
             tc.tile_pool(name="ps", bufs=2, space="PSUM") as ps:
            rec_dram = rec_t
            p0i = big.tile([128, C], dt.int32)
            p1i = big.tile([128, C], dt.int32)
            p0f = big.tile([128, C], dt.float32)
            prow_d_col = big.tile([128, 1], dt.float32)
            nc.sync.dma_start(p0i[:], p0i_d.ap())
            nc.sync.dma_start(p1i[:], p1i_d.ap())
            nc.sync.dma_start(p0f[:], p0f_d.ap())
            nc.sync.dma_start(prow_d_col[:], prow_d.ap())
            rec0f = big.tile([128, C * 8], dt.float32)
            rec1f = big.tile([128, C * 8], dt.float32)
            nc.gpsimd.memset(rec0f[:], 0.0)
            nc.gpsimd.memset(rec1f[:], 0.0)
            for c in range(C):
                nc.gpsimd.indirect_dma_start(
                    rec0f[:, c * 8:(c + 1) * 8], None, rec_dram.ap(),
                    IndirectOffsetOnAxis(ap=p0i[:, c:c + 1], axis=0))
                nc.gpsimd.indirect_dma_start(
                    rec1f[:, c * 8:(c + 1) * 8], None, rec_dram.ap(),
                    IndirectOffsetOnAxis(ap=p1i[:, c:c + 1], axis=0))
            rec0 = rec0f[:].rearrange("p (c e) -> p c e", e=8)
            rec1 = rec1f[:].rearrange("p (c e) -> p c e", e=8)

            # ---- energy per pair slot [128, C]
            d2 = big.tile([128, C], dt.float32)
            tmp = big.tile([128, C], dt.float32)
            nc.gpsimd.memset(d2[:], 0.0)
            for k in range(3):
                dx = sm.tile([128, C], dt.float32, tag="dx")
                nc.vector.tensor_tensor(dx[:], rec0[:, :, k], rec1[:, :, k],
                                        AluOp.subtract)
                nc.vector.tensor_scalar(dx[:], dx[:], 1e-6, None, AluOp.add)
                nc.vector.tensor_tensor(dx[:], dx[:], dx[:], AluOp.mult)
                nc.vector.tensor_tensor(d2[:], d2[:], dx[:], AluOp.add)
            dist = big.tile([128, C], dt.float32)
            nc.scalar.activation(dist[:], d2[:], mybir.ActivationFunctionType.Sqrt)
            padv01 = big.tile([128, C], dt.float32)
            nc.vector.tensor_scalar(padv01[:], p0f[:], float(N_ATOMS) - 0.5,
                                    None, AluOp.is_gt)
            rd = big.tile([128, C], dt.float32)
            nc.vector.tensor_tensor(rd[:], rec0[:, :, 3], rec1[:, :, 3],
                                    AluOp.subtract)
            nc.scalar.activation(rd[:], rd[:], mybir.ActivationFunctionType.Abs)
            nc.vector.tensor_tensor(rd[:], rd[:], padv01[:], AluOp.add)
            lg = big.tile([128, C], dt.float32)
            nc.scalar.activation(lg[:], rd[:], mybir.ActivationFunctionType.Ln)
            netE = big.tile([128, C], dt.float32)
            # netE = 0.5*(-0.298*(2.1 + 2.9823825*lg) + 5*|dist-2.04|)
            nc.vector.tensor_scalar(netE[:], lg[:],
                                    0.5 * -0.001 * TEMPERATURE * 2.9823825,
                                    0.5 * -0.001 * TEMPERATURE * 2.1,
                                    AluOp.mult, AluOp.add)
            nc.vector.tensor_scalar(tmp[:], dist[:], 2.04, None,
                                    AluOp.subtract)
            nc.scalar.activation(tmp[:], tmp[:], mybir.ActivationFunctionType.Abs)
            nc.vector.tensor_scalar(tmp[:], tmp[:], 2.5, None, AluOp.mult)
            nc.vector.tensor_tensor(netE[:], netE[:], tmp[:], AluOp.add)
            # alt bits -> contrib [128, C, 4]
            ab = big.tile([128, C], dt.int32)
            a0i = big.tile([128, C], dt.int32)
            a1i = big.tile([128, C], dt.int32)
            nc.vector.tensor_copy(a0i[:], rec0[:, :, 6])
            nc.vector.tensor_copy(a1i[:], rec1[:, :, 6])
            nc.vector.tensor_tensor(ab[:], a0i[:], a1i[:], AluOp.bitwise_and)
            contrib = big.tile([128, C, 4], dt.float32)
            for k in range(A):
                bk = sm.tile([128, C], dt.int32, tag="bk")
                bf = sm.tile([128, C], dt.float32, tag="bf")
                nc.vector.tensor_scalar(bk[:], ab[:], 1 << k, None,
                                        AluOp.bitwise_and)
                nc.vector.tensor_copy(bf[:], bk[:])
                nc.vector.tensor_scalar(bf[:], bf[:], 0.0, None, AluOp.not_equal)
                nc.vector.tensor_tensor(contrib[:, :, k], netE[:], bf[:],
                                        AluOp.mult)

            # ---- flat offsets (+pad fix: pads have p0f > N_ATOMS)
            # pads write to dump rows NBINS+p (sliced off on host)
            prow = big.tile([128, C], dt.float32)
            nc.vector.tensor_scalar(prow[:], padv01[:], prow_d_col[:], None,
                                    AluOp.mult)
            vm = big.tile([128, C], dt.float32)
            nc.vector.tensor_scalar(vm[:], padv01[:], -1.0, 1.0, AluOp.mult,
                                    AluOp.add)
            # flat = b*200000 + c*50000 + resnum, from gathered b/c/r
            f0 = big.tile([128, C], dt.float32)
            f1 = big.tile([128, C], dt.float32)
            fb = big.tile([128, C], dt.float32)
            for (ft, rc) in ((f0, rec0), (f1, rec1)):
                nc.vector.tensor_scalar(ft[:], rc[:, :, 4], 200000.0, None,
                                        AluOp.mult)
                nc.vector.tensor_scalar(fb[:], rc[:, :, 5], 50000.0, None,
                                        AluOp.mult)
                nc.vector.tensor_tensor(ft[:], ft[:], fb[:], AluOp.add)
                nc.vector.tensor_tensor(ft[:], ft[:], rc[:, :, 3], AluOp.add)
                nc.vector.tensor_tensor(ft[:], ft[:], vm[:], AluOp.mult)
                nc.vector.tensor_tensor(ft[:], ft[:], prow[:], AluOp.add)
            f0i = big.tile([128, C], dt.int32)
            f1i = big.tile([128, C], dt.int32)
            nc.vector.tensor_copy(f0i[:], f0[:])
            nc.vector.tensor_copy(f1i[:], f1[:])

            # ---- dedup totals via per-column group matmuls, then write scatters
            ma0 = big.tile([128, C * 128], dt.float32)
            ma1 = big.tile([128, C * 128], dt.float32)
            mr0 = big.tile([128, C * 128], dt.float32)
            mr1 = big.tile([128, C * 128], dt.float32)
            nc.sync.dma_start(ma0[:], ma0_d.ap())
            nc.sync.dma_start(ma1[:], ma1_d.ap())
            nc.sync.dma_start(mr0[:], mr0_d.ap())
            nc.sync.dma_start(mr1[:], mr1_d.ap())
            dbg = big.tile([128, C * 24], dt.float32)
            nc.vector.tensor_copy(dbg[:].rearrange("p (c e) -> p c e", e=24)[:, :, 0:8], rec0)
            nc.vector.tensor_copy(dbg[:].rearrange("p (c e) -> p c e", e=24)[:, :, 8:16], rec1)
            nc.vector.tensor_copy(dbg[:].rearrange("p (c e) -> p c e", e=24)[:, :, 16:17], netE[:].rearrange("p c -> p c ()"))
            nc.vector.tensor_copy(dbg[:].rearrange("p (c e) -> p c e", e=24)[:, :, 17:21], contrib[:])
            nc.vector.tensor_copy(dbg[:].rearrange("p (c e) -> p c e", e=24)[:, :, 21:22], rd[:].rearrange("p c -> p c ()"))
            nc.vector.tensor_copy(dbg[:].rearrange("p (c e) -> p c e", e=24)[:, :, 22:23], dist[:].rearrange("p c -> p c ()"))
            nc.sync.dma_start(dbg_d.ap(), dbg[:])
            tots = {}
            for name, m in (("a0", ma0), ("a1", ma1), ("r0", mr0), ("r1", mr1)):
                tot = big.tile([128, C * 4], dt.float32, tag="tot" + name)
                pt = ps.tile([128, C * 4], dt.float32, tag="pt" + name)
                for c in range(C):
                    nc.tensor.matmul(out=pt[:, c * 4:(c + 1) * 4],
                                     lhsT=m[:, c * 128:(c + 1) * 128],
                                     rhs=contrib[:, c, :], start=True, stop=True)
                nc.vector.tensor_copy(tot[:], pt[:])
                tots[name] = tot
            for c in range(C):
                nc.gpsimd.indirect_dma_start(
                    ae_d.ap(), IndirectOffsetOnAxis(ap=p0i[:, c:c + 1], axis=0),
                    tots["a0"][:, c * 4:(c + 1) * 4], None)
                nc.gpsimd.indirect_dma_start(
                    ae_d.ap(), IndirectOffsetOnAxis(ap=p1i[:, c:c + 1], axis=0),
                    tots["a1"][:, c * 4:(c + 1) * 4], None)
                nc.gpsimd.indirect_dma_start(
                    re_d.ap(), IndirectOffsetOnAxis(ap=f0i[:, c:c + 1], axis=0),
                    tots["r0"][:, c * 4:(c + 1) * 4], None)
                nc.gpsimd.indirect_dma_start(
                    re_d.ap(), IndirectOffsetOnAxis(ap=f1i[:, c:c + 1], axis=0),
                    tots["r1"][:, c * 4:(c + 1) * 4], None)
    nc.compile()
    return nc


def _get_tail_program():
    if "tail" not in _prog_cache:
        _prog_cache["tail"] = _build_tail_program()
    return _prog_cache["tail"]


class _UF:
    def __init__(self, n):
        self.p = list(range(n))

    def find(self, x):
        while self.p[x] != x:
            self.p[x] = self.p[self.p[x]]
            x = self.p[x]
        return x

    def union(self, a, b):
        ra, rb = self.find(a), self.find(b)
        if ra != rb:
            self.p[ra] = rb


def _pack_tail_core(h0, h1, fl0, fl1):
    """Pack this core's hit pairs into [128, C_PAIR] columns such that no two
    columns share an atom id or flat id. Returns (rows, cols, ok)."""
    n = len(h0)
    C = C_PAIR
    if n == 0:
        return np.zeros(0, np.int64), np.zeros(0, np.int64), True
    uf = _UF(n)
    for keys in (np.concatenate([h0, h1]), np.concatenate([fl0, fl1])):
        pid = np.tile(np.arange(n), 2)
        o = np.argsort(keys, kind="stable")
        ks, po = keys[o], pid[o]
        same = np.flatnonzero(ks[1:] == ks[:-1])
        for i in same:
            uf.union(int(po[i]), int(po[i + 1]))
    root = np.array([uf.find(i) for i in range(n)])
    comps = {}
    for i, r in enumerate(root):
        comps.setdefault(r, []).append(i)
    # first-fit into C columns of capacity 128
    cap = [128] * C
    rows = np.empty(n, np.int64)
    cols = np.empty(n, np.int64)
    for members in sorted(comps.values(), key=len, reverse=True):
        sz = len(members)
        if sz > 128:
            return rows, cols, False
        placed = False
        for c in range(C):
            if cap[c] >= sz:
                r0 = 128 - cap[c]
                for j, m in enumerate(members):
                    rows[m] = r0 + j
                    cols[m] = c
                cap[c] -= sz
                placed = True
                break
        if not placed:
            return rows, cols, False
    return rows, cols, True


def _host_tail(coords, resnum, alt, p0, p1, hit):
    atomEnergy = np.zeros((N_ATOMS, A), np.float32)
    if len(hit):
        h0, h1 = p0[hit], p1[hit]
        diff = coords[h0] - coords[h1] + 1e-6
        dist = np.sqrt(np.sum(diff * diff, axis=-1))
        rd = np.abs(resnum[h0] - resnum[h1]).astype(np.float32)
        energy = -0.001 * TEMPERATURE * (2.1 + 2.9823825 * np.log(rd)) \
            + 5.0 * np.abs(dist - 2.04)
        netE = 0.5 * energy
        pair_alt = alt[h0] & alt[h1]
        contrib = np.where(pair_alt, netE[:, None], 0.0).astype(np.float32)
        np.add.at(atomEnergy, h0, contrib)
        np.add.at(atomEnergy, h1, contrib)
    return atomEnergy


def kernel(coords, atom_description, atom_number, atomPairs, alternativeMask,
           partners, facc):
    coords = np.asarray(coords, np.float32)
    ad = np.asarray(atom_description)
    pairs = np.asarray(atomPairs).astype(np.int64)
    alt = np.asarray(alternativeMask).astype(bool)
    at_name = ad[:, 3].astype(np.int64)
    resnum = ad[:, 2].astype(np.int64)
    p0, p1 = pairs[:, 0], pairs[:, 1]
    npair = len(p0)

    at_u8 = at_name.astype(np.uint8)

    # choose T adaptively (worst-case chunk count over both sides)
    def n_chunks_needed(p):
        win_counts = np.bincount(p // WIN)
        wc = win_counts[win_counts > 0]
        return int(np.sum((wc + F - 1) // F))

    need = max(n_chunks_needed(p0), n_chunks_needed(p1))
    T = max(1, -(-need // (8 * 128)))
    nc1 = _get_program(T, 1)
    nc2 = _get_program(T, 2)

    # weights for bitmap packing
    k = np.arange(32)
    w_lo = np.where(k < 16, 2.0 ** k, 0.0)
    w_hi = np.where(k >= 16, 2.0 ** (k - 16), 0.0)
    wts = np.tile((w_lo + w_hi).astype(np.float32), T)[None, :].repeat(128, 0)
    wts[:, 31::32] = 0.0

    # ---- phase 1: p1 side
    slot1, pw1, boc1, _ = _build_layout(p1, T)
    anw1 = _build_anw(at_u8, boc1)
    res1 = _run_phase(nc1, pw1, anw1, wts)
    m1 = np.stack([res1.results[c]["mby"] for c in range(N_CORES)])  # [8,128,TF]
    m1_of_pair = m1.reshape(-1)[slot1]

    # ---- relay into p0 layout ({0,1} int8; phase 2 fuses ne + logical_and)
    slot0, pw0, boc0, _ = _build_layout(p0, T)
    anw0 = _build_anw(at_u8, boc0)
    mfac0 = np.zeros(8 * 128 * T * F, np.int8)
    mfac0[slot0] = m1_of_pair
    res0 = _run_phase(nc2, pw0, anw0, wts,
                      mfac0.reshape(8, 128, T * F))
    sby = np.stack([res0.results[c]["mby"] for c in range(N_CORES)])
    sulfur = sby.reshape(-1)[slot0] != 0

    # ---- sparse tail on device (phase 3); host fallback for over-capacity
    flat = ((ad[:, 0].astype(np.int64) * N_CHAIN + ad[:, 1]) * N_RES + resnum)
    hit = np.flatnonzero(sulfur)
    # Device tail by default; host fallback only for over-capacity inputs
    # (>128*C_PAIR sulfur pairs per core) or if explicitly disabled.
    import os
    use_device_tail = (os.environ.get("DISULF_DEVICE_TAIL", "1") == "1"
                       and len(hit) <= N_CORES * 128 * C_PAIR)

    if use_device_tail:
        from concourse import bass_utils
        nct = _get_tail_program()
        C = C_PAIR
        # host-assembled record table (pure relayout of input columns):
        # [x, y, z, resnum, batch, chain, altbits, 0]
        NP_ = 128 * REC_P
        rec_in = np.zeros((NP_, 8), np.float32)
        rec_in[:N_ATOMS, 0:3] = coords
        rec_in[:N_ATOMS, 3] = resnum
        rec_in[:N_ATOMS, 4] = ad[:, 0]
        rec_in[:N_ATOMS, 5] = ad[:, 1]
        rec_in[:N_ATOMS, 6] = (alt * (1 << np.arange(4))).sum(1)
        # split hits evenly across cores
        in_maps = []
        per = [hit[c::N_CORES] for c in range(N_CORES)]
        ok_all = True
        for c in range(N_CORES):
            hc = per[c]
            h0, h1 = p0[hc], p1[hc]
            f0, f1 = flat[h0], flat[h1]
            rows, cols, ok = _pack_tail_core(h0, h1, f0, f1)
            ok_all &= ok
            if not ok:
                break
            rowpad = (N_ATOMS + np.arange(128, dtype=np.int32))[:, None]
            p0i = np.broadcast_to(rowpad, (128, C)).astype(np.int32).copy()
            p1i = p0i.copy()
            # pad ids N_ATOMS+p: never equal a real atom; within a column all distinct
            a0 = p0i.astype(np.int64).copy()
            a1 = a0.copy()
            g0 = np.broadcast_to(NBINS + np.arange(128)[:, None],
                                 (128, C)).astype(np.int64).copy()
            g1 = g0.copy()
            a0[rows, cols] = h0
            a1[rows, cols] = h1
            g0[rows, cols] = f0
            g1[rows, cols] = f1
            p0i[rows, cols] = h0
            p1i[rows, cols] = h1
            ma0 = np.zeros((128, C, 128), np.float32)
            ma1 = np.zeros((128, C, 128), np.float32)
            mr0 = np.zeros((128, C, 128), np.float32)
            mr1 = np.zeros((128, C, 128), np.float32)
            for cc in range(C):
                A0, A1 = a0[:, cc], a1[:, cc]
                G0, G1 = g0[:, cc], g1[:, cc]
                ma0[:, cc, :] = ((A0[:, None] == A0[None, :]) +
                                 (A1[:, None] == A0[None, :])).astype(np.float32)
                ma1[:, cc, :] = ((A0[:, None] == A1[None, :]) +
                                 (A1[:, None] == A1[None, :])).astype(np.float32)
                mr0[:, cc, :] = ((G0[:, None] == G0[None, :]) +
                                 (G1[:, None] == G0[None, :])).astype(np.float32)
                mr1[:, cc, :] = ((G0[:, None] == G1[None, :]) +
                                 (G1[:, None] == G1[None, :])).astype(np.float32)
            in_maps.append(dict(
                rect=rec_in,
                p0i=p0i, p1i=p1i, p0f=p0i.astype(np.float32),
                ma0=ma0.reshape(128, -1), ma1=ma1.reshape(128, -1),
                mr0=mr0.reshape(128, -1), mr1=mr1.reshape(128, -1),
                prow=(NBINS + np.arange(128, dtype=np.float32))[:, None]))
        if ok_all:
            rest = bass_utils.run_bass_kernel_spmd(
                nct, in_maps, core_ids=list(range(N_CORES)))
            atomEnergy = np.zeros((N_ATOMS, A), np.float32)
            residueEnergy = np.zeros((NBINS, A), np.float32)
            for c in range(N_CORES):
                atomEnergy += rest.results[c]["ae"][:N_ATOMS]
                residueEnergy += rest.results[c]["re"][:NBINS]
        else:
            use_device_tail = False

    if not use_device_tail:
        atomEnergy = _host_tail(coords, resnum, alt, p0, p1, hit)
        residueEnergy = np.zeros((NBINS, A), np.float32)
        nz = np.flatnonzero(np.any(atomEnergy != 0, axis=1))
        np.add.at(residueEnergy, flat[nz], atomEnergy[nz])

    residueEnergy = residueEnergy.reshape(N_BATCH, N_CHAIN, N_RES, A)
    return residueEnergy, atomEnergy, sulfur
